# revision 54
# baseline (speedup 1.0000x reference)
"""Trainium2 Bass kernel for CirculantMultiHeadAttention.

Strategy
--------
Host side: the block-circulant weights (4,4,512) are materialized into dense
(2048,2048) matrices, because on TRN2 a dense matmul on the PE array beats any
FFT formulation by a wide margin (the FFT's pointwise stage would swamp the
vector engines).  Work is sharded over the 8 NeuronCores as (batch b in {0,1})
x (head-group g in {0..3}, 4 heads each): core c = 4*b + g.  Each core
computes q/k/v projections for its 4 heads, RoPE, causal attention, and a
*partial* output projection (contracting only its own 512 context features).
The host sums the 4 partials per batch.

Device side (per core, one Bass program, SPMD over 8 cores):
  - the big contractions (q/k/v projections over D=2048, output projection
    over 512 ctx features) run as THREE fp8-e4m3 DoubleRow chains per psum:
    W8@x8 + W8@xr + Wr@x8, where W8/x8 are e4m3 quantizations and Wr/xr the
    e4m3-quantized residuals.  DoubleRow contracts 2 k-tiles per matmul at
    0.5 PE cycles/row, so the 3-chain hi/lo split costs 0.75x the bf16
    cycles while the residual cancellation keeps bf16-class accuracy
    (~0.35% max rel err end-to-end vs the 2e-2 budget).  Scale bookkeeping:
    weights x64 (e4m3 normal range), folded back in the fp16 RoPE tables
    (q/k), the ctx eviction x0.25 (v path), and the out eviction x1/1024.
  - S = q.k stays bf16 (a 128-deep hd contraction cannot pair DoubleRow
    k-tiles without a partition-shifting eviction, and single-fp8 operands
    would put ~5% noise on scores); P = exp(S) stays bf16 (a second exp
    pass for a residual would double the ACT-engine load).  PV and the
    denominator path therefore stay bf16 except the PV/outproj operands
    above.
  - attention in scores-transposed layout: S_T[k, q] = k_tile.T @ q_chunk,
    P_T = exp(S_T * scale) on ScalarE, causal masking only on the single
    mixed 128x128 corner of each diagonal tile (the rest is all-valid),
    PV accumulation ctxT[d, q] += v_tile.T @ P_T, denominators as a bf16
    pairwise partial-sum chain on DVE (second level + last-chunk work on
    Pool, which must never touch PSUM on real HW).
  - ctx is evicted normalized (x 0.25/den) to bf16, then quantized to the
    e4m3 hi/lo pair tiles that feed the output-projection DoubleRow chains.
  - software pipeline: chunk-0 projections alone, then chunk i projections
    interleaved ~1:1.7 with chunk i-1 attention, then last-chunk attention
    with output-projection psums as PE filler, then a drain of the last 4
    output tiles in two-tile g0/g1 half-waves (borrowing the idle S/ctx
    psum banks) so heads-0/1 matmuls cover the last ctx-eviction latency.
  - engine budget at 232us: PE 225.6us (97%), DVE ~150, ACT ~155, Pool ~75.
    PE work: proj 123.3 + S 29 + PV 29 + den-free + outproj 41 (all us).
"""

import os
import sys

import numpy as np

for _p in ("/opt/trn_rl_repo", "/root/.axon_site/_ro/trn_rl_repo"):
    if os.path.isdir(_p) and _p not in sys.path:
        sys.path.insert(0, _p)

import ml_dtypes

import concourse.bass as bass
import concourse.tile as tile
from concourse import bacc, bass_isa, mybir
from concourse.bass_utils import run_bass_kernel_spmd

F32 = mybir.dt.float32
F16 = mybir.dt.float16
AF = mybir.ActivationFunctionType

# Problem geometry (hardcoded per spec).
B, T_FULL, D = 2, 2048, 2048
H, HD = 16, 128
NCORES = 8
HG = 4                    # heads per core
FS = HG * HD              # 512 feature dims per core
P = 128                   # partitions
KT = D // P               # 16 contraction tiles for projections
SCALE = 1.0 / float(np.sqrt(HD))
MASKW = 896               # triangular mask strip width: 512 + 3*128

# Matmul operand dtype.  bfloat16: 1 cycle/row at any moving width on the PE
# (fp32r needs >=256-wide or pays 4x), half the DMA/SBUF of fp32.  HW/sim
# end-to-end relative error ~1e-3 vs the 2e-2 budget.  CIRC_MM_DT=float32r
# restores the TF32-like mode.
MM_DT = os.environ.get("CIRC_MM_DT", "bfloat16")

# q/k/v projections run as THREE fp8(e4m3) DoubleRow chains per psum:
#   W8@x8 + W8@xr + Wr@x8   (W8 = e4m3(64*W), Wr = e4m3(64*W - W8),
#                            x8 = e4m3(x),    xr = e4m3(x - x8))
# DoubleRow contracts 2 k-tiles per matmul at 0.5 PE cycles/row, so the
# 3-chain split runs at 0.75x the bf16 cycle cost with bf16-class accuracy
# (hi/lo residual cancellation; verified ~0.3% max rel err end to end).
# The 64x weight scale keeps e4m3 mantissas in the normal range; it is
# divided back out in the RoPE tables (q/k) and in the host-side w_o (v).
WSCALE = 64.0
NPAIR = KT // 2           # 8 DoubleRow k-tile pairs per contraction


def _mm_dt():
    return getattr(mybir.dt, MM_DT)


def _np_dt():
    return ml_dtypes.bfloat16 if MM_DT == "bfloat16" else np.float32


# ---------------------------------------------------------------------------
# Device program
# ---------------------------------------------------------------------------

def _body(es, tc, io, T):
    nc = tc.nc
    ntc = T // 512            # t-chunks of 512
    nkt = T // P              # 128-wide t/k tiles
    mdt = _mm_dt()
    E4 = mybir.dt.float8e4
    DR = mybir.MatmulPerfMode.DoubleRow

    x8d, xrd, wq8d, wqrd, wk8d, wkrd, wv8d, wvrd, \
        wo8d, word, cos2, sin2, maskR, out = io
    OSC = 1.0 / (16.0 * WSCALE)     # ctx 16x * wo 64x, folded at out evict

    # ---- persistent SBUF tiles ------------------------------------------
    const = es.enter_context(tc.tile_pool(name="const", bufs=1))
    mask_sb = const.tile([P, MASKW], mdt, tag="maskR", name="mask_sb")

    # q/k stay SBUF-resident across phases in [feat, t] layout, one tile per
    # head; v in [t, feat] tiles.  No DRAM bounce.
    qkp = es.enter_context(tc.tile_pool(name="qkall", bufs=HG))
    q_all = [qkp.tile([P, T], mdt, tag="qall", name="q_all") for _ in range(HG)]
    k_all = [qkp.tile([P, T], mdt, tag="kall", name="k_all") for _ in range(HG)]
    vap = es.enter_context(tc.tile_pool(name="vall", bufs=nkt))
    v_all = [None] * nkt

    # output-projection operands in fp8 hi/lo pair layout: ctx as two
    # head-pair tiles [P, 2, T] per variant, w_o as [P, 2, D] per pair group
    wop = es.enter_context(tc.tile_pool(name="wo", bufs=4))
    wo8_sb = [wop.tile([P, 2, D], E4, tag="wo", name="wo8_sb")
              for _ in range(2)]
    wor_sb = [wop.tile([P, 2, D], E4, tag="wo", name="wor_sb")
              for _ in range(2)]
    ctxp = es.enter_context(tc.tile_pool(name="ctx", bufs=HG))
    cx8_sb = [ctxp.tile([P, 2, T], E4, tag="ctx", name="cx8_sb")
              for _ in range(2)]
    cxr_sb = [ctxp.tile([P, 2, T], E4, tag="ctx", name="cxr_sb")
              for _ in range(2)]

    with (
        tc.tile_pool(name="wq", bufs=1) as wqp,
        tc.tile_pool(name="wk", bufs=1) as wkp,
        tc.tile_pool(name="wv", bufs=1) as wvp,
        tc.tile_pool(name="xt", bufs=4) as xtp,
        tc.tile_pool(name="pev", bufs=2) as evp,
        tc.tile_pool(name="trig", bufs=2) as trigp,
        tc.tile_pool(name="pT", bufs=8) as pTp,
        tc.tile_pool(name="pacc", bufs=2) as paccp,
        tc.tile_pool(name="amisc", bufs=2) as amp,
        tc.tile_pool(name="oev", bufs=7) as oevp,
        tc.tile_pool(name="pps", bufs=4, space="PSUM") as psp,
        tc.tile_pool(name="sps", bufs=2, space="PSUM") as sps,
        tc.tile_pool(name="cps", bufs=2, space="PSUM") as cps,
    ):
        # ---- input DMAs: x + wv on SP, wq + wo + mask on Pool (gpsimd),
        # cos/sin + wk on Activation, so the v-projection weights never
        # queue behind wk and the PE can start on x[0]/wq[0] immediately.
        # fp8 streams are packed host-side as [128, pair, 2, cols] so one
        # DMA fills a whole chunk/weight tile in DoubleRow layout.
        # first chunk + first weights split in halves so the first psum
        # chain can start on pairs 0-3 while pairs 4-7 are still in flight
        x8_first = xtp.tile([P, NPAIR, 2, 512], E4, tag="xt", name="x8_sb")
        xr_first = xtp.tile([P, NPAIR, 2, 512], E4, tag="xt", name="xr_sb")
        qp = NPAIR // 4
        for q_i in range(4):
            nc.sync.dma_start(out=x8_first[:, q_i * qp:(q_i + 1) * qp],
                              in_=x8d[:, 0, q_i * qp:(q_i + 1) * qp])
        wq_sb = [wqp.tile([P, NPAIR, 2, FS], E4, tag="wq", name="wq_sb",
                          bufs=2) for _ in range(2)]
        wk_sb = [wkp.tile([P, NPAIR, 2, FS], E4, tag="wk", name="wk_sb",
                          bufs=2) for _ in range(2)]
        wv_sb = [wvp.tile([P, NPAIR, 2, FS], E4, tag="wv", name="wv_sb",
                          bufs=2) for _ in range(2)]
        for q_i in range(4):
            nc.gpsimd.dma_start(out=wq_sb[0][:, q_i * qp:(q_i + 1) * qp],
                                in_=wq8d[:, q_i * qp:(q_i + 1) * qp])
        nc.gpsimd.dma_start(out=wq_sb[1][:], in_=wqrd[:])
        trig_sb = [None] * ntc
        cos_sb0 = trigp.tile([P, 512], F16, tag="cos", name="cos_sb")
        nc.scalar.dma_start(out=cos_sb0[:], in_=cos2[:, 0:512])
        sin_sb0 = trigp.tile([P, 512], F16, tag="sin", name="sin_sb")
        nc.scalar.dma_start(out=sin_sb0[:], in_=sin2[:, 0:512])
        trig_sb[0] = (cos_sb0, sin_sb0)
        nc.scalar.dma_start(out=xr_first[:], in_=xrd[:, 0])
        nc.scalar.dma_start(out=wk_sb[0][:], in_=wk8d[:])
        nc.sync.dma_start(out=wk_sb[1][:], in_=wkrd[:])
        nc.sync.dma_start(out=wv_sb[0][:], in_=wv8d[:])
        nc.sync.dma_start(out=wv_sb[1][:], in_=wvrd[:])
        for g in range(2):
            nc.gpsimd.dma_start(out=wo8_sb[g][:], in_=wo8d[g])
            nc.gpsimd.dma_start(out=wor_sb[g][:], in_=word[g])
        nc.gpsimd.dma_start(out=mask_sb[:], in_=maskR[:, :])

        # ---- emitter builders -------------------------------------------
        def proj_chunk_emitters(tci):
            """12 closures: 8 q/k head-projections (fused RoPE) + 4 v."""
            tsl = slice(tci * 512, (tci + 1) * 512)
            if tci == 0:
                x8_sb, xr_sb = x8_first, xr_first
            else:
                x8_sb = xtp.tile([P, NPAIR, 2, 512], E4, tag="xt",
                                 name="x8_sb")
                xr_sb = xtp.tile([P, NPAIR, 2, 512], E4, tag="xt",
                                 name="xr_sb")

            def prefetch():
                if tci > 0:
                    cos_sb = trigp.tile([P, 512], F16, tag="cos",
                                        name="cos_sb")
                    nc.gpsimd.dma_start(out=cos_sb[:], in_=cos2[:, tsl])
                    sin_sb = trigp.tile([P, 512], F16, tag="sin",
                                        name="sin_sb")
                    nc.gpsimd.dma_start(out=sin_sb[:], in_=sin2[:, tsl])
                    trig_sb[tci] = (cos_sb, sin_sb)
                    nc.sync.dma_start(out=x8_sb[:], in_=x8d[:, tci])
                    nc.gpsimd.dma_start(out=xr_sb[:], in_=xrd[:, tci])

            def qk_em(wsb, dst, h):
                def em():
                    cos_sb, sin_sb = trig_sb[tci]
                    hsl = slice(h * P, (h + 1) * P)
                    ps = psp.tile([P, 512], F32, tag="ps", name="ps")
                    chains = ((wsb[0], x8_sb), (wsb[0], xr_sb),
                              (wsb[1], x8_sb))
                    for ci, (wt, xt) in enumerate(chains):
                        for m in range(NPAIR):
                            nc.tensor.matmul(
                                ps[:], wt[:, m, :, hsl], xt[:, m, :, :],
                                start=(ci == 0 and m == 0),
                                stop=(ci == 2 and m == NPAIR - 1),
                                perf_mode=DR)
                    # RoPE: rot = [-odd; even] of ps (ACT, the psum read),
                    # then all-16-bit DVE ops at the 2x_1p rate
                    rot = evp.tile([P, 512], mdt, tag="rot", name="rot")
                    nc.scalar.mul(rot[0:64, :], ps[64:128, :], -1.0)
                    nc.scalar.copy(rot[64:128, :], ps[0:64, :])
                    o = evp.tile([P, 512], mdt, tag="o", name="o")
                    nc.vector.tensor_mul(o[:], ps[:], cos_sb[:])
                    nc.vector.tensor_mul(rot[:], rot[:], sin_sb[:])
                    nc.vector.tensor_add(dst[h][:, tsl], o[:], rot[:])
                return em

            def v_em(ts):
                def em():
                    tt = tci * 4 + ts
                    psl = slice(ts * P, (ts + 1) * P)
                    ps = psp.tile([P, FS], F32, tag="ps", name="ps")
                    chains = ((x8_sb, wv_sb[0]), (xr_sb, wv_sb[0]),
                              (x8_sb, wv_sb[1]))
                    for ci, (xt, wt) in enumerate(chains):
                        for m in range(NPAIR):
                            nc.tensor.matmul(
                                ps[:], xt[:, m, :, psl], wt[:, m, :, :],
                                start=(ci == 0 and m == 0),
                                stop=(ci == 2 and m == NPAIR - 1),
                                perf_mode=DR)
                    vt = vap.tile([P, FS], mdt, tag="vall", name="v_all")
                    nc.vector.tensor_copy(vt[:], ps[:])
                    v_all[tt] = vt
                return em

            ems = []
            for wsb, dst in ((wq_sb, q_all), (wk_sb, k_all)):
                for h in range(HG):
                    ems.append(qk_em(wsb, dst, h))
            for ts in range(4):
                ems.append(v_em(ts))
            return prefetch, ems

        drain_ps = {}

        def outproj_psum(tt, ncj, final=False, evict_act=False, half=None):
            # one psum group of 6 fp8 DoubleRow matmuls (hi*hi + lo*hi +
            # hi*lo chains x 2 head-pair groups, ~640ns of dep-free PE
            # work) -- the filler currency interleaved into the attention.
            # half=0/1 emits only the g=0 / g=1 chain halves (drain waves).
            nsl = slice(ncj * 512, (ncj + 1) * 512)
            tsl = slice(tt * P, (tt + 1) * P)
            if half == 1:
                ps = drain_ps.pop((tt, ncj))
            elif half == 0 and tt % 2 == 1:
                # odd drain tiles borrow the idle S/ctx psum banks so two
                # tiles' g0 waves can run ahead of the last ctx eviction
                pool = sps if ncj < 2 else cps
                ps = pool.tile([P, 512], F32,
                               tag="sps" if ncj < 2 else "cps", name="ops")
            else:
                ps = psp.tile([P, 512], F32, tag="ps", name="ops")
            chains = ((cx8_sb, wo8_sb), (cxr_sb, wo8_sb), (cx8_sb, wor_sb))
            order = [(c, g) for g in range(2) for c in ((0, 2, 1) if g
                                                        else (0, 1, 2))]
            if half is not None:
                order = [(c, g) for c, g in order if g == half]
            for n_i, (ci, g) in enumerate(order):
                cx, wo_v = chains[ci]
                nc.tensor.matmul(ps[:], cx[g][:, :, tsl],
                                 wo_v[g][:, :, nsl],
                                 start=(half in (None, 0) and n_i == 0),
                                 stop=(half in (None, 1) and
                                       n_i == len(order) - 1),
                                 perf_mode=DR)
            if half == 0:
                drain_ps[(tt, ncj)] = ps
                return
            o = oevp.tile([P, 512], mdt, tag="o", name="o")
            if final and ncj % 2:
                nc.scalar.mul(o[:], ps[:], OSC)
            else:
                nc.vector.tensor_scalar_mul(o[:], ps[:], OSC)
            eng = nc.gpsimd if ncj % 2 else nc.sync
            eng.dma_start(out=out[tsl, nsl], in_=o[:])

        def attn_head_emitters(h, qc):
            """nmg+1 closures; micro-group i = S+exp for kt pair i, with
            the masked PV + denominator accumulation pipelined one step
            behind.  PE filler (outproj of tile ftt) is woven in before the
            early S pairs; ftt is shifted back one tile so the h=0 head of
            each chunk fills with a tile whose ctx is long finished."""
            qsl = slice(qc * 512, (qc + 1) * 512)
            nk = 4 * (qc + 1)
            nmg = nk // 2
            hsl = slice(h * P, (h + 1) * P)
            ftt = 4 * (qc - 1) + h - 1
            fillers = [(ftt, j) for j in range(4)] if ftt >= 0 else []
            if qc == ntc - 1 and h == HG - 1:
                # last head also covers tile ftt+1 so the post-attention
                # drain only has 4 tiles left
                fillers += [(ftt + 1, j) for j in range(4)]
            st = {}

            def tile_slices(kt):
                j = kt - 4 * qc
                c0 = 128 * j if j > 0 else 0
                return slice(qc * 512 + c0, (qc + 1) * 512), slice(c0, 512), c0

            def s_pair(i):
                for kt in (2 * i, 2 * i + 1):
                    lsl, psl, c0 = tile_slices(kt)
                    s_ps = sps.tile([P, 512], F32, tag="sps", name="s_ps")
                    nc.tensor.matmul(s_ps[:, psl],
                                     k_all[h][:, kt * P:(kt + 1) * P],
                                     q_all[h][:, lsl], start=True, stop=True)
                    p_t = pTp.tile([P, 512], mdt, tag="pT", name="p_t")
                    nc.scalar.activation(p_t[:, psl], s_ps[:, psl], AF.Exp,
                                         scale=SCALE)
                    st[kt] = (p_t, None)

            def pv_pair(i):
                kts = (2 * i, 2 * i + 1)
                for kt in kts:
                    _, psl, c0 = tile_slices(kt)
                    pt, _ = st[kt]
                    if kt >= 4 * qc:
                        # only the first 128 cols of a diagonal tile mix
                        # valid/invalid; beyond them every row is valid
                        nc.gpsimd.tensor_mul(pt[:, c0:c0 + P],
                                             pt[:, c0:c0 + P],
                                             mask_sb[:, 384:384 + P])
                    nc.tensor.matmul(st["ctx"][:, psl], v_all[kt][:, hsl],
                                     pt[:, psl],
                                     start=(kt == 0), stop=(kt == nk - 1))
                # denominator accumulation (off the PE): full-width pairs
                # stay in a bf16 partial-sum chain at 2x DVE rate; partial
                # tiles and the flush go through the fp32 accumulator
                k0, k1 = kts
                full = k1 < 4 * qc + 1   # both tiles full 512 wide
                pt0, _ = st[k0]
                pt1, _ = st[k1]
                if full and k0 > 0:
                    t1 = pTp.tile([P, 512], mdt, tag="ds", name="t1",
                                  bufs=4)
                    t1eng = nc.gpsimd if qc == ntc - 1 else nc.vector
                    t1eng.tensor_add(t1[:], pt0[:], pt1[:])
                    if st.get("dsum") is None:
                        st["dsum"] = t1
                    else:
                        t2 = pTp.tile([P, 512], mdt, tag="ds", name="t2",
                                      bufs=4)
                        nc.gpsimd.tensor_add(t2[:], st["dsum"][:], t1[:])
                        st["dsum"] = t2
                else:
                    for kt in kts:
                        _, psl, _ = tile_slices(kt)
                        pt, _ = st[kt]
                        if kt == 0:
                            nc.vector.tensor_copy(st["pacc"][:], pt[:])
                        else:
                            nc.vector.tensor_add(st["pacc"][:, psl],
                                                 st["pacc"][:, psl],
                                                 pt[:, psl])
                for kt in kts:
                    del st[kt]

            def em_i(i):
                def em():
                    if i == 0:
                        st["ctx"] = cps.tile([P, 512], F32, tag="cps",
                                             name="ctx_ps")
                        st["pacc"] = paccp.tile([P, 512], F32, tag="pacc",
                                                name="pacc")
                    if i < nmg:
                        if fillers and i < nmg - 1:
                            rem = max(nmg - 1 - i, 1)
                            nf = (len(fillers) + rem - 1) // rem
                            for _ in range(nf):
                                tt_f, j_f = fillers.pop(0)
                                outproj_psum(tt_f, j_f)
                        s_pair(i)
                    if i > 0:
                        pv_pair(i - 1)
                    last = qc == ntc - 1 and h == HG - 1
                    if i == nmg - 1 and last and st.get("dsum") is not None:
                        # the dsum chain is complete one micro-step early;
                        # flushing here keeps it off the drain critical path
                        nc.vector.tensor_add(st["pacc"][:], st["pacc"][:],
                                             st["dsum"][:])
                        st["dsum"] = None
                    if i == nmg:
                        if st.get("dsum") is not None:
                            nc.vector.tensor_add(st["pacc"][:],
                                                 st["pacc"][:],
                                                 st["dsum"][:])
                            st["dsum"] = None
                        rs_red = amp.tile([P, 512], F32, tag="rs",
                                          name="rs_red")
                        nc.gpsimd.partition_all_reduce(
                            rs_red[:], st["pacc"][:], channels=P,
                            reduce_op=bass_isa.ReduceOp.add)
                        nc.vector.reciprocal(rs_red[:], rs_red[:])
                        # cbf = 0.25 * ctx_psum / den  (16x true scale --
                        # keeps the e4m3 hi part clear of the 240 ceiling)
                        g, blk = divmod(h, 2)
                        # for the very last head, evict in two half-width
                        # pipelined pieces so the output-projection drain
                        # can start on the first piece sooner
                        parts = ((slice(0, 256), slice(qc * 512,
                                                       qc * 512 + 256)),
                                 (slice(256, 512), slice(qc * 512 + 256,
                                                         (qc + 1) * 512))
                                 ) if last else ((slice(0, 512), qsl),)
                        cbf = amp.tile([P, 512], mdt, tag="cbf", name="cbf")
                        for psl_c, qsl_c in parts:
                            nc.vector.scalar_tensor_tensor(
                                cbf[:, psl_c], st["ctx"][:, psl_c], 0.25,
                                rs_red[:, psl_c],
                                mybir.AluOpType.mult, mybir.AluOpType.mult)
                            nc.gpsimd.tensor_copy(
                                cx8_sb[g][:, blk, qsl_c], cbf[:, psl_c])
                            sube = nc.vector if last else nc.gpsimd
                            sube.tensor_sub(cxr_sb[g][:, blk, qsl_c],
                                            cbf[:, psl_c],
                                            cx8_sb[g][:, blk, qsl_c])
                return em
            return [em_i(i) for i in range(nmg + 1)]

        # ---- schedule ----------------------------------------------------
        # software pipeline: chunk-0 projections alone, then chunk tci's
        # projections interleaved with chunk tci-1's attention, then the
        # last chunk's attention alone.  This spreads the attention's
        # DVE/ACT load (exp, masks, denominators) across the whole
        # timeline instead of saturating those engines after the
        # projections finish.
        pfs, emss = [], []
        for tci in range(ntc):
            pf, ems = proj_chunk_emitters(tci)
            pfs.append(pf)
            emss.append(ems)
        for i, em in enumerate(emss[0]):
            if i == 4:
                pfs[1]()    # chunk-1 x/trig DMAs fire during chunk 0
            em()
        for tci in range(1, ntc):
            pe_ems = emss[tci]
            at_ems = [em for h in range(HG)
                      for em in attn_head_emitters(h, tci - 1)]
            npe, na = len(pe_ems), len(at_ems)
            ipe = ia = 0
            while ipe < npe or ia < na:
                if ipe < npe and (ia >= na or ipe * na <= ia * npe):
                    if ipe == 6 and tci + 1 < ntc:
                        pfs[tci + 1]()
                    pe_ems[ipe]()
                    ipe += 1
                else:
                    at_ems[ia]()
                    ia += 1
        for h in range(HG):
            for em in attn_head_emitters(h, ntc - 1):
                em()
        # final output projection: tiles 12..15 in two-tile waves of
        # g0-then-g1 halves so up to 8 psums of head-0/1 work run while
        # the last head's ctx eviction is still in flight
        t0 = 4 * (ntc - 1)
        for ta in (t0, t0 + 2):
            for tt in (ta, ta + 1):
                for ncj in range(4):
                    outproj_psum(tt, ncj, final=True, half=0)
            for tt in (ta, ta + 1):
                for ncj in range(4):
                    outproj_psum(tt, ncj, final=True, half=1)


def build_program(T=T_FULL):
    from contextlib import ExitStack

    nc = bacc.Bacc("TRN2", target_bir_lowering=False, debug=False,
                   num_devices=NCORES)
    mdt = _mm_dt()
    E4 = mybir.dt.float8e4
    ntc = T // 512
    x8d = nc.dram_tensor("x8", (P, ntc, NPAIR, 2, 512), E4,
                         kind="ExternalInput").ap()
    xrd = nc.dram_tensor("xr", (P, ntc, NPAIR, 2, 512), E4,
                         kind="ExternalInput").ap()
    wts = {}
    for wn in ("wq8", "wqr", "wk8", "wkr", "wv8", "wvr"):
        wts[wn] = nc.dram_tensor(wn, (P, NPAIR, 2, FS), E4,
                                 kind="ExternalInput").ap()
    wo8d = nc.dram_tensor("wo8", (2, P, 2, D), E4, kind="ExternalInput").ap()
    word = nc.dram_tensor("wor", (2, P, 2, D), E4, kind="ExternalInput").ap()
    cos2 = nc.dram_tensor("cos2", (P, T), F16, kind="ExternalInput").ap()
    sin2 = nc.dram_tensor("sin2", (P, T), F16, kind="ExternalInput").ap()
    maskR = nc.dram_tensor("maskR", (P, MASKW), mdt,
                           kind="ExternalInput").ap()
    out = nc.dram_tensor("out", (T, D), mdt, kind="ExternalOutput").ap()

    io = (x8d, xrd, wts["wq8"], wts["wqr"], wts["wk8"], wts["wkr"],
          wts["wv8"], wts["wvr"], wo8d, word, cos2, sin2, maskR, out)
    with tile.TileContext(nc) as tc:
        with ExitStack() as es:
            _body(es, tc, io, T)
    nc.compile()
    return nc


# ---------------------------------------------------------------------------
# Host-side data prep
# ---------------------------------------------------------------------------

def dense_from_circulant(w):
    """(qb, pb, bs) generating vectors -> dense (qb*bs, pb*bs) matrix."""
    w = np.asarray(w, dtype=np.float32)
    qb, pb, bs = w.shape
    idx = (np.arange(bs)[:, None] - np.arange(bs)[None, :]) % bs
    blocks = w[:, :, idx]                      # (qb, pb, bs, bs)
    return np.ascontiguousarray(
        blocks.transpose(0, 2, 1, 3).reshape(qb * bs, pb * bs))


_EO_PERM = np.concatenate([np.arange(0, HD, 2), np.arange(1, HD, 2)])


def _perm_rows_even_odd(w_rows):
    """Permute each 128-row head block to (even rows, odd rows)."""
    nh = w_rows.shape[0] // HD
    blocks = w_rows.reshape(nh, HD, -1)[:, _EO_PERM, :]
    return blocks.reshape(w_rows.shape)


def rope_tables(T=T_FULL, theta=10000.0):
    # 1/WSCALE folds the fp8 weight scale back out of the q/k psums
    inv = 1.0 / (theta ** (np.arange(0, HD, 2, dtype=np.float32) / HD))
    ang = np.arange(T, dtype=np.float32)[:, None] * inv[None, :]
    cos = (np.cos(ang) / WSCALE).astype(np.float16).T      # (64, T)
    sin = (np.sin(ang) / WSCALE).astype(np.float16).T
    cos2 = np.ascontiguousarray(np.concatenate([cos, cos], axis=0))
    sin2 = np.ascontiguousarray(np.concatenate([sin, sin], axis=0))
    return cos2, sin2


E4NP = ml_dtypes.float8_e4m3


def _pack_x_fp8(xT):
    """xT (D, T) fp32 -> (x8, xr) packed [128, ntc, NPAIR, 2, 512] e4m3."""
    x8 = xT.astype(E4NP)
    xr = (xT - x8.astype(np.float32)).astype(E4NP)
    ntc = xT.shape[1] // 512

    def pack(a):
        # d = 256*m + 128*b + p ; t = 512*c + j
        a = a.reshape(NPAIR, 2, P, ntc, 512).transpose(2, 3, 0, 1, 4)
        return np.ascontiguousarray(a)
    return pack(x8), pack(xr)


def _pack_w_fp8(wT):
    """wT (D, FS) fp32 -> (w8, wr) packed [128, NPAIR, 2, FS] e4m3."""
    ws = wT * WSCALE
    w8 = ws.astype(E4NP)
    wr = (ws - w8.astype(np.float32)).astype(E4NP)

    def pack(a):
        a = a.reshape(NPAIR, 2, P, FS).transpose(2, 0, 1, 3)
        return np.ascontiguousarray(a)
    return pack(w8), pack(wr)


def _pack_wo_fp8(woT):
    """woT (FS, D) fp32 -> (wo8, wor) packed [2, 128, 2, D] e4m3.
    Pair group g holds heads (2g, 2g+1) as DoubleRow contraction subtiles."""
    ws = woT * WSCALE
    w8 = ws.astype(E4NP)
    wr = (ws - w8.astype(np.float32)).astype(E4NP)

    def pack(a):
        a = a.reshape(2, 2, P, D).transpose(0, 2, 1, 3)
        return np.ascontiguousarray(a)
    return pack(w8), pack(wr)


def mask_strip():
    kk = np.arange(P)[:, None]
    c = np.arange(MASKW)[None, :]
    return np.ascontiguousarray(((c - 384) >= kk).astype(np.float32))


def make_in_maps(x, w_q, w_k, w_v, w_o, T=T_FULL):
    """Build the 8 per-core input maps from full inputs."""
    ndt = _np_dt()
    x = np.asarray(x, dtype=np.float32)
    Wq = dense_from_circulant(w_q)
    Wk = dense_from_circulant(w_k)
    Wv = dense_from_circulant(w_v)
    Wo = dense_from_circulant(w_o)
    cos2, sin2 = rope_tables(T)
    mstrip = mask_strip()

    xpk = [_pack_x_fp8(np.ascontiguousarray(x[b, :T, :].T))
           for b in range(B)]
    wpk = {}
    for g in range(NCORES // B):
        fs = slice(FS * g, FS * (g + 1))
        wpk[g] = {
            "wq": _pack_w_fp8(
                np.ascontiguousarray(_perm_rows_even_odd(Wq[fs, :]).T)),
            "wk": _pack_w_fp8(
                np.ascontiguousarray(_perm_rows_even_odd(Wk[fs, :]).T)),
            "wv": _pack_w_fp8(np.ascontiguousarray(Wv[fs, :].T)),
            "wo": _pack_wo_fp8(np.ascontiguousarray(Wo[:, fs].T)),
        }
    in_maps = []
    for c in range(NCORES):
        b, g = divmod(c, NCORES // B)
        w = wpk[g]
        in_maps.append({
            "x8": xpk[b][0],
            "xr": xpk[b][1],
            "wq8": w["wq"][0], "wqr": w["wq"][1],
            "wk8": w["wk"][0], "wkr": w["wk"][1],
            "wv8": w["wv"][0], "wvr": w["wv"][1],
            "wo8": w["wo"][0], "wor": w["wo"][1],
            "cos2": cos2,
            "sin2": sin2,
            "maskR": mstrip.astype(ndt),
        })
    return in_maps


_PROGRAM_CACHE = {}


def get_program(T=T_FULL):
    key = (T, MM_DT)
    if key not in _PROGRAM_CACHE:
        _PROGRAM_CACHE[key] = build_program(T)
    return _PROGRAM_CACHE[key]


LAST_EXEC_NS = None


def kernel(x, w_q, w_k, w_v, w_o, mask=None, trace=False):
    """Full inputs in, full output out.  Shards over 8 NeuronCores."""
    global LAST_EXEC_NS
    x = np.asarray(x, dtype=np.float32)
    in_maps = make_in_maps(x, w_q, w_k, w_v, w_o, T_FULL)
    nc = get_program(T_FULL)
    try:
        res = run_bass_kernel_spmd(nc, in_maps, core_ids=list(range(NCORES)),
                                   trace=trace)
    except ModuleNotFoundError:
        # no NTFF profiling hook in this container; run untraced
        res = run_bass_kernel_spmd(nc, in_maps, core_ids=list(range(NCORES)),
                                   trace=False)
    LAST_EXEC_NS = res.exec_time_ns
    gpb = NCORES // B
    out = np.stack([
        sum(np.asarray(res.results[b * gpb + g]["out"], dtype=np.float64)
            for g in range(gpb)).astype(np.float32)
        for b in range(B)
    ])
    return out



# revision 58
# speedup vs baseline: 1.0039x; 1.0039x over previous
"""Trainium2 Bass kernel for CirculantMultiHeadAttention.

Strategy
--------
Host side: the block-circulant weights (4,4,512) are materialized into dense
(2048,2048) matrices, because on TRN2 a dense matmul on the PE array beats any
FFT formulation by a wide margin (the FFT's pointwise stage would swamp the
vector engines).  Work is sharded over the 8 NeuronCores as (batch b in {0,1})
x (head-group g in {0..3}, 4 heads each): core c = 4*b + g.  Each core
computes q/k/v projections for its 4 heads, RoPE, causal attention, and a
*partial* output projection (contracting only its own 512 context features).
The host sums the 4 partials per batch.

Device side (per core, one Bass program, SPMD over 8 cores):
  - the big contractions (q/k/v projections over D=2048, output projection
    over 512 ctx features) run as THREE fp8-e4m3 DoubleRow chains per psum:
    W8@x8 + W8@xr + Wr@x8, where W8/x8 are e4m3 quantizations and Wr/xr the
    e4m3-quantized residuals.  DoubleRow contracts 2 k-tiles per matmul at
    0.5 PE cycles/row, so the 3-chain hi/lo split costs 0.75x the bf16
    cycles while the residual cancellation keeps bf16-class accuracy
    (~0.35% max rel err end-to-end vs the 2e-2 budget).  Scale bookkeeping:
    weights x64 (e4m3 normal range), folded back in the fp16 RoPE tables
    (q/k), the ctx eviction x0.25 (v path), and the out eviction x1/1024.
  - S = q.k stays bf16 (a 128-deep hd contraction cannot pair DoubleRow
    k-tiles without a partition-shifting eviction, and single-fp8 operands
    would put ~5% noise on scores); P = exp(S) stays bf16 (a second exp
    pass for a residual would double the ACT-engine load).  PV and the
    denominator path therefore stay bf16 except the PV/outproj operands
    above.
  - attention in scores-transposed layout: S_T[k, q] = k_tile.T @ q_chunk,
    P_T = exp(S_T * scale) on ScalarE, causal masking only on the single
    mixed 128x128 corner of each diagonal tile (the rest is all-valid),
    PV accumulation ctxT[d, q] += v_tile.T @ P_T, denominators as a bf16
    pairwise partial-sum chain on DVE (second level + last-chunk work on
    Pool, which must never touch PSUM on real HW).
  - ctx is evicted normalized (x 0.25/den) to bf16, then quantized to the
    e4m3 hi/lo pair tiles that feed the output-projection DoubleRow chains.
  - software pipeline: chunk-0 projections alone, then chunk i projections
    interleaved ~1:1.7 with chunk i-1 attention, then last-chunk attention
    with output-projection psums as PE filler, then a drain of the last 4
    output tiles in two-tile g0/g1 half-waves (borrowing the idle S/ctx
    psum banks) so heads-0/1 matmuls cover the last ctx-eviction latency.
  - engine budget at 232us: PE 225.6us (97%), DVE ~150, ACT ~155, Pool ~75.
    PE work: proj 123.3 + S 29 + PV 29 + den-free + outproj 41 (all us).
"""

import os
import sys

import numpy as np

for _p in ("/opt/trn_rl_repo", "/root/.axon_site/_ro/trn_rl_repo"):
    if os.path.isdir(_p) and _p not in sys.path:
        sys.path.insert(0, _p)

import ml_dtypes

import concourse.bass as bass
import concourse.tile as tile
from concourse import bacc, bass_isa, mybir
from concourse.bass_utils import run_bass_kernel_spmd

F32 = mybir.dt.float32
F16 = mybir.dt.float16
AF = mybir.ActivationFunctionType

# Problem geometry (hardcoded per spec).
B, T_FULL, D = 2, 2048, 2048
H, HD = 16, 128
NCORES = 8
HG = 4                    # heads per core
FS = HG * HD              # 512 feature dims per core
P = 128                   # partitions
KT = D // P               # 16 contraction tiles for projections
SCALE = 1.0 / float(np.sqrt(HD))
MASKW = 896               # triangular mask strip width: 512 + 3*128

# Matmul operand dtype.  bfloat16: 1 cycle/row at any moving width on the PE
# (fp32r needs >=256-wide or pays 4x), half the DMA/SBUF of fp32.  HW/sim
# end-to-end relative error ~1e-3 vs the 2e-2 budget.  CIRC_MM_DT=float32r
# restores the TF32-like mode.
MM_DT = os.environ.get("CIRC_MM_DT", "bfloat16")

# q/k/v projections run as THREE fp8(e4m3) DoubleRow chains per psum:
#   W8@x8 + W8@xr + Wr@x8   (W8 = e4m3(64*W), Wr = e4m3(64*W - W8),
#                            x8 = e4m3(x),    xr = e4m3(x - x8))
# DoubleRow contracts 2 k-tiles per matmul at 0.5 PE cycles/row, so the
# 3-chain split runs at 0.75x the bf16 cycle cost with bf16-class accuracy
# (hi/lo residual cancellation; verified ~0.3% max rel err end to end).
# The 64x weight scale keeps e4m3 mantissas in the normal range; it is
# divided back out in the RoPE tables (q/k) and in the host-side w_o (v).
WSCALE = 64.0
NPAIR = KT // 2           # 8 DoubleRow k-tile pairs per contraction


def _mm_dt():
    return getattr(mybir.dt, MM_DT)


def _np_dt():
    return ml_dtypes.bfloat16 if MM_DT == "bfloat16" else np.float32


# ---------------------------------------------------------------------------
# Device program
# ---------------------------------------------------------------------------

def _body(es, tc, io, T):
    nc = tc.nc
    ntc = T // 512            # t-chunks of 512
    nkt = T // P              # 128-wide t/k tiles
    mdt = _mm_dt()
    E4 = mybir.dt.float8e4
    DR = mybir.MatmulPerfMode.DoubleRow

    x8d, xrd, wq8d, wqrd, wk8d, wkrd, wv8d, wvrd, \
        wo8d, word, cos2, sin2, maskR, out = io
    OSC = 1.0 / (16.0 * WSCALE)     # ctx 16x * wo 64x, folded at out evict

    # ---- persistent SBUF tiles ------------------------------------------
    const = es.enter_context(tc.tile_pool(name="const", bufs=1))
    mask_sb = const.tile([P, MASKW], mdt, tag="maskR", name="mask_sb")

    # q/k stay SBUF-resident across phases in [feat, t] layout, one tile per
    # head; v in [t, feat] tiles.  No DRAM bounce.
    qkp = es.enter_context(tc.tile_pool(name="qkall", bufs=HG))
    q_all = [qkp.tile([P, T], mdt, tag="qall", name="q_all") for _ in range(HG)]
    k_all = [qkp.tile([P, T], mdt, tag="kall", name="k_all") for _ in range(HG)]
    vap = es.enter_context(tc.tile_pool(name="vall", bufs=nkt))
    v_all = [None] * nkt

    # output-projection operands in fp8 hi/lo pair layout: ctx as two
    # head-pair tiles [P, 2, T] per variant, w_o as [P, 2, D] per pair group
    wop = es.enter_context(tc.tile_pool(name="wo", bufs=4))
    wo8_sb = [wop.tile([P, 2, D], E4, tag="wo", name="wo8_sb")
              for _ in range(2)]
    wor_sb = [wop.tile([P, 2, D], E4, tag="wo", name="wor_sb")
              for _ in range(2)]
    ctxp = es.enter_context(tc.tile_pool(name="ctx", bufs=HG))
    cx8_sb = [ctxp.tile([P, 2, T], E4, tag="ctx", name="cx8_sb")
              for _ in range(2)]
    cxr_sb = [ctxp.tile([P, 2, T], E4, tag="ctx", name="cxr_sb")
              for _ in range(2)]

    with (
        tc.tile_pool(name="wq", bufs=1) as wqp,
        tc.tile_pool(name="wk", bufs=1) as wkp,
        tc.tile_pool(name="wv", bufs=1) as wvp,
        tc.tile_pool(name="xt", bufs=4) as xtp,
        tc.tile_pool(name="pev", bufs=2) as evp,
        tc.tile_pool(name="trig", bufs=2) as trigp,
        tc.tile_pool(name="pT", bufs=8) as pTp,
        tc.tile_pool(name="pacc", bufs=2) as paccp,
        tc.tile_pool(name="amisc", bufs=2) as amp,
        tc.tile_pool(name="oev", bufs=7) as oevp,
        tc.tile_pool(name="pps", bufs=4, space="PSUM") as psp,
        tc.tile_pool(name="sps", bufs=2, space="PSUM") as sps,
        tc.tile_pool(name="cps", bufs=2, space="PSUM") as cps,
    ):
        # ---- input DMAs: x + wv on SP, wq + wo + mask on Pool (gpsimd),
        # cos/sin + wk on Activation, so the v-projection weights never
        # queue behind wk and the PE can start on x[0]/wq[0] immediately.
        # fp8 streams are packed host-side as [128, pair, 2, cols] so one
        # DMA fills a whole chunk/weight tile in DoubleRow layout.
        # first chunk + first weights split in halves so the first psum
        # chain can start on pairs 0-3 while pairs 4-7 are still in flight
        x8_first = xtp.tile([P, NPAIR, 2, 512], E4, tag="xt", name="x8_sb")
        xr_first = xtp.tile([P, NPAIR, 2, 512], E4, tag="xt", name="xr_sb")
        qp = NPAIR // 4
        for q_i in range(4):
            nc.sync.dma_start(out=x8_first[:, q_i * qp:(q_i + 1) * qp],
                              in_=x8d[:, 0, q_i * qp:(q_i + 1) * qp])
        wq_sb = [wqp.tile([P, NPAIR, 2, FS], E4, tag="wq", name="wq_sb",
                          bufs=2) for _ in range(2)]
        wk_sb = [wkp.tile([P, NPAIR, 2, FS], E4, tag="wk", name="wk_sb",
                          bufs=2) for _ in range(2)]
        wv_sb = [wvp.tile([P, NPAIR, 2, FS], E4, tag="wv", name="wv_sb",
                          bufs=2) for _ in range(2)]
        for q_i in range(4):
            nc.gpsimd.dma_start(out=wq_sb[0][:, q_i * qp:(q_i + 1) * qp],
                                in_=wq8d[:, q_i * qp:(q_i + 1) * qp])
        nc.gpsimd.dma_start(out=wq_sb[1][:], in_=wqrd[:])
        trig_sb = [None] * ntc
        cos_sb0 = trigp.tile([P, 512], F16, tag="cos", name="cos_sb")
        nc.scalar.dma_start(out=cos_sb0[:], in_=cos2[:, 0:512])
        sin_sb0 = trigp.tile([P, 512], F16, tag="sin", name="sin_sb")
        nc.scalar.dma_start(out=sin_sb0[:], in_=sin2[:, 0:512])
        trig_sb[0] = (cos_sb0, sin_sb0)
        nc.scalar.dma_start(out=xr_first[:], in_=xrd[:, 0])
        nc.scalar.dma_start(out=wk_sb[0][:], in_=wk8d[:])
        nc.sync.dma_start(out=wk_sb[1][:], in_=wkrd[:])
        nc.sync.dma_start(out=wv_sb[0][:], in_=wv8d[:])
        nc.sync.dma_start(out=wv_sb[1][:], in_=wvrd[:])
        for g in range(2):
            nc.gpsimd.dma_start(out=wo8_sb[g][:], in_=wo8d[g])
            nc.gpsimd.dma_start(out=wor_sb[g][:], in_=word[g])
        nc.gpsimd.dma_start(out=mask_sb[:], in_=maskR[:, :])

        # ---- emitter builders -------------------------------------------
        def proj_chunk_emitters(tci):
            """12 closures: 8 q/k head-projections (fused RoPE) + 4 v."""
            tsl = slice(tci * 512, (tci + 1) * 512)
            if tci == 0:
                x8_sb, xr_sb = x8_first, xr_first
            else:
                x8_sb = xtp.tile([P, NPAIR, 2, 512], E4, tag="xt",
                                 name="x8_sb")
                xr_sb = xtp.tile([P, NPAIR, 2, 512], E4, tag="xt",
                                 name="xr_sb")

            def prefetch():
                if tci > 0:
                    cos_sb = trigp.tile([P, 512], F16, tag="cos",
                                        name="cos_sb")
                    nc.gpsimd.dma_start(out=cos_sb[:], in_=cos2[:, tsl])
                    sin_sb = trigp.tile([P, 512], F16, tag="sin",
                                        name="sin_sb")
                    nc.gpsimd.dma_start(out=sin_sb[:], in_=sin2[:, tsl])
                    trig_sb[tci] = (cos_sb, sin_sb)
                    nc.sync.dma_start(out=x8_sb[:], in_=x8d[:, tci])
                    nc.gpsimd.dma_start(out=xr_sb[:], in_=xrd[:, tci])

            def qk_em(wsb, dst, h):
                def em():
                    cos_sb, sin_sb = trig_sb[tci]
                    hsl = slice(h * P, (h + 1) * P)
                    ps = psp.tile([P, 512], F32, tag="ps", name="ps")
                    chains = ((wsb[0], x8_sb), (wsb[0], xr_sb),
                              (wsb[1], x8_sb))
                    for ci, (wt, xt) in enumerate(chains):
                        for m in range(NPAIR):
                            nc.tensor.matmul(
                                ps[:], wt[:, m, :, hsl], xt[:, m, :, :],
                                start=(ci == 0 and m == 0),
                                stop=(ci == 2 and m == NPAIR - 1),
                                perf_mode=DR)
                    # RoPE: rot = [-odd; even] of ps (ACT, the psum read),
                    # then all-16-bit DVE ops at the 2x_1p rate
                    rot = evp.tile([P, 512], mdt, tag="rot", name="rot")
                    nc.scalar.mul(rot[0:64, :], ps[64:128, :], -1.0)
                    nc.scalar.copy(rot[64:128, :], ps[0:64, :])
                    o = evp.tile([P, 512], mdt, tag="o", name="o")
                    nc.vector.tensor_mul(o[:], ps[:], cos_sb[:])
                    nc.vector.tensor_mul(rot[:], rot[:], sin_sb[:])
                    nc.vector.tensor_add(dst[h][:, tsl], o[:], rot[:])
                return em

            def v_em(ts):
                def em():
                    tt = tci * 4 + ts
                    psl = slice(ts * P, (ts + 1) * P)
                    ps = psp.tile([P, FS], F32, tag="ps", name="ps")
                    chains = ((x8_sb, wv_sb[0]), (xr_sb, wv_sb[0]),
                              (x8_sb, wv_sb[1]))
                    for ci, (xt, wt) in enumerate(chains):
                        for m in range(NPAIR):
                            nc.tensor.matmul(
                                ps[:], xt[:, m, :, psl], wt[:, m, :, :],
                                start=(ci == 0 and m == 0),
                                stop=(ci == 2 and m == NPAIR - 1),
                                perf_mode=DR)
                    vt = vap.tile([P, FS], mdt, tag="vall", name="v_all")
                    nc.vector.tensor_copy(vt[:], ps[:])
                    v_all[tt] = vt
                return em

            ems = []
            for wsb, dst in ((wq_sb, q_all), (wk_sb, k_all)):
                for h in range(HG):
                    ems.append(qk_em(wsb, dst, h))
            for ts in range(4):
                ems.append(v_em(ts))
            return prefetch, ems

        drain_ps = {}

        def outproj_psum(tt, ncj, final=False, evict_act=False, half=None):
            # one psum group of 6 fp8 DoubleRow matmuls (hi*hi + lo*hi +
            # hi*lo chains x 2 head-pair groups, ~640ns of dep-free PE
            # work) -- the filler currency interleaved into the attention.
            # half=0/1 emits only the g=0 / g=1 chain halves (drain waves).
            nsl = slice(ncj * 512, (ncj + 1) * 512)
            tsl = slice(tt * P, (tt + 1) * P)
            if half == 1:
                ps = drain_ps.pop((tt, ncj))
            elif half == 0 and tt % 2 == 1:
                # odd drain tiles borrow the idle S/ctx psum banks so two
                # tiles' g0 waves can run ahead of the last ctx eviction
                pool = sps if ncj < 2 else cps
                ps = pool.tile([P, 512], F32,
                               tag="sps" if ncj < 2 else "cps", name="ops")
            else:
                ps = psp.tile([P, 512], F32, tag="ps", name="ops")
            chains = ((cx8_sb, wo8_sb), (cxr_sb, wo8_sb), (cx8_sb, wor_sb))
            order = [(c, g) for g in range(2) for c in ((0, 2, 1) if g
                                                        else (0, 1, 2))]
            if half is not None:
                order = [(c, g) for c, g in order if g == half]
            for n_i, (ci, g) in enumerate(order):
                cx, wo_v = chains[ci]
                nc.tensor.matmul(ps[:], cx[g][:, :, tsl],
                                 wo_v[g][:, :, nsl],
                                 start=(half in (None, 0) and n_i == 0),
                                 stop=(half in (None, 1) and
                                       n_i == len(order) - 1),
                                 perf_mode=DR)
            if half == 0:
                drain_ps[(tt, ncj)] = ps
                return
            o = oevp.tile([P, 512], mdt, tag="o", name="o")
            if final and ncj % 2:
                nc.scalar.mul(o[:], ps[:], OSC)
            else:
                nc.vector.tensor_scalar_mul(o[:], ps[:], OSC)
            eng = nc.gpsimd if ncj % 2 else nc.sync
            eng.dma_start(out=out[tsl, nsl], in_=o[:])

        def attn_head_emitters(h, qc):
            """nmg+1 closures; micro-group i = S+exp for kt pair i, with
            the masked PV + denominator accumulation pipelined one step
            behind.  PE filler (outproj of tile ftt) is woven in before the
            early S pairs; ftt is shifted back one tile so the h=0 head of
            each chunk fills with a tile whose ctx is long finished."""
            qsl = slice(qc * 512, (qc + 1) * 512)
            nk = 4 * (qc + 1)
            nmg = nk // 2
            hsl = slice(h * P, (h + 1) * P)
            ftt = 4 * (qc - 1) + h - 1
            fillers = [(ftt, j) for j in range(4)] if ftt >= 0 else []
            if qc == ntc - 1 and h == HG - 1:
                # last head also covers tile ftt+1 so the post-attention
                # drain only has 4 tiles left
                fillers += [(ftt + 1, j) for j in range(4)]
            st = {}

            def tile_slices(kt):
                j = kt - 4 * qc
                c0 = 128 * j if j > 0 else 0
                return slice(qc * 512 + c0, (qc + 1) * 512), slice(c0, 512), c0

            def s_pair(i):
                for kt in (2 * i, 2 * i + 1):
                    lsl, psl, c0 = tile_slices(kt)
                    s_ps = sps.tile([P, 512], F32, tag="sps", name="s_ps")
                    nc.tensor.matmul(s_ps[:, psl],
                                     k_all[h][:, kt * P:(kt + 1) * P],
                                     q_all[h][:, lsl], start=True, stop=True)
                    p_t = pTp.tile([P, 512], mdt, tag="pT", name="p_t")
                    nc.scalar.activation(p_t[:, psl], s_ps[:, psl], AF.Exp,
                                         scale=SCALE)
                    st[kt] = (p_t, None)

            def pv_pair(i):
                kts = (2 * i, 2 * i + 1)
                for kt in kts:
                    _, psl, c0 = tile_slices(kt)
                    pt, _ = st[kt]
                    if kt >= 4 * qc:
                        # only the first 128 cols of a diagonal tile mix
                        # valid/invalid; beyond them every row is valid
                        nc.gpsimd.tensor_mul(pt[:, c0:c0 + P],
                                             pt[:, c0:c0 + P],
                                             mask_sb[:, 384:384 + P])
                    nc.tensor.matmul(st["ctx"][:, psl], v_all[kt][:, hsl],
                                     pt[:, psl],
                                     start=(kt == 0), stop=(kt == nk - 1))
                # denominator accumulation (off the PE): full-width pairs
                # stay in a bf16 partial-sum chain at 2x DVE rate; partial
                # tiles and the flush go through the fp32 accumulator
                k0, k1 = kts
                full = k1 < 4 * qc + 1   # both tiles full 512 wide
                pt0, _ = st[k0]
                pt1, _ = st[k1]
                if full and k0 > 0:
                    t1 = pTp.tile([P, 512], mdt, tag="ds", name="t1",
                                  bufs=4)
                    t1eng = nc.gpsimd if qc == ntc - 1 else nc.vector
                    t1eng.tensor_add(t1[:], pt0[:], pt1[:])
                    if st.get("dsum") is None:
                        st["dsum"] = t1
                    else:
                        t2 = pTp.tile([P, 512], mdt, tag="ds", name="t2",
                                      bufs=4)
                        nc.gpsimd.tensor_add(t2[:], st["dsum"][:], t1[:])
                        st["dsum"] = t2
                else:
                    for kt in kts:
                        _, psl, _ = tile_slices(kt)
                        pt, _ = st[kt]
                        if kt == 0:
                            nc.vector.tensor_copy(st["pacc"][:], pt[:])
                        else:
                            nc.vector.tensor_add(st["pacc"][:, psl],
                                                 st["pacc"][:, psl],
                                                 pt[:, psl])
                for kt in kts:
                    del st[kt]

            def em_i(i):
                def em():
                    if i == 0:
                        st["ctx"] = cps.tile([P, 512], F32, tag="cps",
                                             name="ctx_ps")
                        st["pacc"] = paccp.tile([P, 512], F32, tag="pacc",
                                                name="pacc")
                    if i < nmg:
                        if fillers and i < nmg - 1:
                            rem = max(nmg - 1 - i, 1)
                            nf = (len(fillers) + rem - 1) // rem
                            for _ in range(nf):
                                tt_f, j_f = fillers.pop(0)
                                outproj_psum(tt_f, j_f)
                        s_pair(i)
                    if i > 0:
                        pv_pair(i - 1)
                    last = qc == ntc - 1 and h == HG - 1
                    if i == nmg - 1 and last:
                        if st.get("dsum") is not None:
                            # dsum is complete one micro-step early; flush
                            # now to keep it off the drain critical path
                            nc.vector.tensor_add(st["pacc"][:],
                                                 st["pacc"][:],
                                                 st["dsum"][:])
                            st["dsum"] = None
                        # pacc cols [0:256) got their final (pair nmg-2)
                        # contribution already -- the pair nmg-1 diagonal
                        # adds only touch [256:512).  Reduce and recip the
                        # first half early, off the drain critical path.
                        rs_early = amp.tile([P, 512], F32, tag="rs",
                                            name="rs_red")
                        nc.gpsimd.partition_all_reduce(
                            rs_early[:, 0:256], st["pacc"][:, 0:256],
                            channels=P, reduce_op=bass_isa.ReduceOp.add)
                        nc.vector.reciprocal(rs_early[:, 0:256],
                                             rs_early[:, 0:256])
                        st["rs_early"] = rs_early
                    if i == nmg:
                        if st.get("dsum") is not None:
                            nc.vector.tensor_add(st["pacc"][:],
                                                 st["pacc"][:],
                                                 st["dsum"][:])
                            st["dsum"] = None
                        if st.get("rs_early") is not None:
                            rs_red = st.pop("rs_early")
                            nc.gpsimd.partition_all_reduce(
                                rs_red[:, 256:512], st["pacc"][:, 256:512],
                                channels=P, reduce_op=bass_isa.ReduceOp.add)
                            nc.vector.reciprocal(rs_red[:, 256:512],
                                                 rs_red[:, 256:512])
                        else:
                            rs_red = amp.tile([P, 512], F32, tag="rs",
                                              name="rs_red")
                            nc.gpsimd.partition_all_reduce(
                                rs_red[:], st["pacc"][:], channels=P,
                                reduce_op=bass_isa.ReduceOp.add)
                            nc.vector.reciprocal(rs_red[:], rs_red[:])
                        # cbf = 0.25 * ctx_psum / den  (16x true scale --
                        # keeps the e4m3 hi part clear of the 240 ceiling)
                        g, blk = divmod(h, 2)
                        # for the very last head, evict in two half-width
                        # pipelined pieces so the output-projection drain
                        # can start on the first piece sooner
                        parts = ((slice(0, 256), slice(qc * 512,
                                                       qc * 512 + 256)),
                                 (slice(256, 512), slice(qc * 512 + 256,
                                                         (qc + 1) * 512))
                                 ) if last else ((slice(0, 512), qsl),)
                        cbf = amp.tile([P, 512], mdt, tag="cbf", name="cbf")
                        for psl_c, qsl_c in parts:
                            nc.vector.scalar_tensor_tensor(
                                cbf[:, psl_c], st["ctx"][:, psl_c], 0.25,
                                rs_red[:, psl_c],
                                mybir.AluOpType.mult, mybir.AluOpType.mult)
                            nc.gpsimd.tensor_copy(
                                cx8_sb[g][:, blk, qsl_c], cbf[:, psl_c])
                            sube = nc.vector if last else nc.gpsimd
                            sube.tensor_sub(cxr_sb[g][:, blk, qsl_c],
                                            cbf[:, psl_c],
                                            cx8_sb[g][:, blk, qsl_c])
                return em
            return [em_i(i) for i in range(nmg + 1)]

        # ---- schedule ----------------------------------------------------
        # software pipeline: chunk-0 projections alone, then chunk tci's
        # projections interleaved with chunk tci-1's attention, then the
        # last chunk's attention alone.  This spreads the attention's
        # DVE/ACT load (exp, masks, denominators) across the whole
        # timeline instead of saturating those engines after the
        # projections finish.
        pfs, emss = [], []
        for tci in range(ntc):
            pf, ems = proj_chunk_emitters(tci)
            pfs.append(pf)
            emss.append(ems)
        for i, em in enumerate(emss[0]):
            if i == 4:
                pfs[1]()    # chunk-1 x/trig DMAs fire during chunk 0
            em()
        for tci in range(1, ntc):
            pe_ems = emss[tci]
            at_ems = [em for h in range(HG)
                      for em in attn_head_emitters(h, tci - 1)]
            npe, na = len(pe_ems), len(at_ems)
            ipe = ia = 0
            while ipe < npe or ia < na:
                if ipe < npe and (ia >= na or ipe * na <= ia * npe):
                    if ipe == 6 and tci + 1 < ntc:
                        pfs[tci + 1]()
                    pe_ems[ipe]()
                    ipe += 1
                else:
                    at_ems[ia]()
                    ia += 1
        for h in range(HG):
            for em in attn_head_emitters(h, ntc - 1):
                em()
        # final output projection: tiles 12..15 in two-tile waves of
        # g0-then-g1 halves so up to 8 psums of head-0/1 work run while
        # the last head's ctx eviction is still in flight
        t0 = 4 * (ntc - 1)
        for ta in (t0, t0 + 2):
            for tt in (ta, ta + 1):
                for ncj in range(4):
                    outproj_psum(tt, ncj, final=True, half=0)
            for tt in (ta, ta + 1):
                for ncj in range(4):
                    outproj_psum(tt, ncj, final=True, half=1)


def build_program(T=T_FULL):
    from contextlib import ExitStack

    nc = bacc.Bacc("TRN2", target_bir_lowering=False, debug=False,
                   num_devices=NCORES)
    mdt = _mm_dt()
    E4 = mybir.dt.float8e4
    ntc = T // 512
    x8d = nc.dram_tensor("x8", (P, ntc, NPAIR, 2, 512), E4,
                         kind="ExternalInput").ap()
    xrd = nc.dram_tensor("xr", (P, ntc, NPAIR, 2, 512), E4,
                         kind="ExternalInput").ap()
    wts = {}
    for wn in ("wq8", "wqr", "wk8", "wkr", "wv8", "wvr"):
        wts[wn] = nc.dram_tensor(wn, (P, NPAIR, 2, FS), E4,
                                 kind="ExternalInput").ap()
    wo8d = nc.dram_tensor("wo8", (2, P, 2, D), E4, kind="ExternalInput").ap()
    word = nc.dram_tensor("wor", (2, P, 2, D), E4, kind="ExternalInput").ap()
    cos2 = nc.dram_tensor("cos2", (P, T), F16, kind="ExternalInput").ap()
    sin2 = nc.dram_tensor("sin2", (P, T), F16, kind="ExternalInput").ap()
    maskR = nc.dram_tensor("maskR", (P, MASKW), mdt,
                           kind="ExternalInput").ap()
    out = nc.dram_tensor("out", (T, D), mdt, kind="ExternalOutput").ap()

    io = (x8d, xrd, wts["wq8"], wts["wqr"], wts["wk8"], wts["wkr"],
          wts["wv8"], wts["wvr"], wo8d, word, cos2, sin2, maskR, out)
    with tile.TileContext(nc) as tc:
        with ExitStack() as es:
            _body(es, tc, io, T)
    nc.compile()
    return nc


# ---------------------------------------------------------------------------
# Host-side data prep
# ---------------------------------------------------------------------------

def dense_from_circulant(w):
    """(qb, pb, bs) generating vectors -> dense (qb*bs, pb*bs) matrix."""
    w = np.asarray(w, dtype=np.float32)
    qb, pb, bs = w.shape
    idx = (np.arange(bs)[:, None] - np.arange(bs)[None, :]) % bs
    blocks = w[:, :, idx]                      # (qb, pb, bs, bs)
    return np.ascontiguousarray(
        blocks.transpose(0, 2, 1, 3).reshape(qb * bs, pb * bs))


_EO_PERM = np.concatenate([np.arange(0, HD, 2), np.arange(1, HD, 2)])


def _perm_rows_even_odd(w_rows):
    """Permute each 128-row head block to (even rows, odd rows)."""
    nh = w_rows.shape[0] // HD
    blocks = w_rows.reshape(nh, HD, -1)[:, _EO_PERM, :]
    return blocks.reshape(w_rows.shape)


def rope_tables(T=T_FULL, theta=10000.0):
    # 1/WSCALE folds the fp8 weight scale back out of the q/k psums
    inv = 1.0 / (theta ** (np.arange(0, HD, 2, dtype=np.float32) / HD))
    ang = np.arange(T, dtype=np.float32)[:, None] * inv[None, :]
    cos = (np.cos(ang) / WSCALE).astype(np.float16).T      # (64, T)
    sin = (np.sin(ang) / WSCALE).astype(np.float16).T
    cos2 = np.ascontiguousarray(np.concatenate([cos, cos], axis=0))
    sin2 = np.ascontiguousarray(np.concatenate([sin, sin], axis=0))
    return cos2, sin2


E4NP = ml_dtypes.float8_e4m3


def _pack_x_fp8(xT):
    """xT (D, T) fp32 -> (x8, xr) packed [128, ntc, NPAIR, 2, 512] e4m3."""
    x8 = xT.astype(E4NP)
    xr = (xT - x8.astype(np.float32)).astype(E4NP)
    ntc = xT.shape[1] // 512

    def pack(a):
        # d = 256*m + 128*b + p ; t = 512*c + j
        a = a.reshape(NPAIR, 2, P, ntc, 512).transpose(2, 3, 0, 1, 4)
        return np.ascontiguousarray(a)
    return pack(x8), pack(xr)


def _pack_w_fp8(wT):
    """wT (D, FS) fp32 -> (w8, wr) packed [128, NPAIR, 2, FS] e4m3."""
    ws = wT * WSCALE
    w8 = ws.astype(E4NP)
    wr = (ws - w8.astype(np.float32)).astype(E4NP)

    def pack(a):
        a = a.reshape(NPAIR, 2, P, FS).transpose(2, 0, 1, 3)
        return np.ascontiguousarray(a)
    return pack(w8), pack(wr)


def _pack_wo_fp8(woT):
    """woT (FS, D) fp32 -> (wo8, wor) packed [2, 128, 2, D] e4m3.
    Pair group g holds heads (2g, 2g+1) as DoubleRow contraction subtiles."""
    ws = woT * WSCALE
    w8 = ws.astype(E4NP)
    wr = (ws - w8.astype(np.float32)).astype(E4NP)

    def pack(a):
        a = a.reshape(2, 2, P, D).transpose(0, 2, 1, 3)
        return np.ascontiguousarray(a)
    return pack(w8), pack(wr)


def mask_strip():
    kk = np.arange(P)[:, None]
    c = np.arange(MASKW)[None, :]
    return np.ascontiguousarray(((c - 384) >= kk).astype(np.float32))


def make_in_maps(x, w_q, w_k, w_v, w_o, T=T_FULL):
    """Build the 8 per-core input maps from full inputs."""
    ndt = _np_dt()
    x = np.asarray(x, dtype=np.float32)
    Wq = dense_from_circulant(w_q)
    Wk = dense_from_circulant(w_k)
    Wv = dense_from_circulant(w_v)
    Wo = dense_from_circulant(w_o)
    cos2, sin2 = rope_tables(T)
    mstrip = mask_strip()

    xpk = [_pack_x_fp8(np.ascontiguousarray(x[b, :T, :].T))
           for b in range(B)]
    wpk = {}
    for g in range(NCORES // B):
        fs = slice(FS * g, FS * (g + 1))
        wpk[g] = {
            "wq": _pack_w_fp8(
                np.ascontiguousarray(_perm_rows_even_odd(Wq[fs, :]).T)),
            "wk": _pack_w_fp8(
                np.ascontiguousarray(_perm_rows_even_odd(Wk[fs, :]).T)),
            "wv": _pack_w_fp8(np.ascontiguousarray(Wv[fs, :].T)),
            "wo": _pack_wo_fp8(np.ascontiguousarray(Wo[:, fs].T)),
        }
    in_maps = []
    for c in range(NCORES):
        b, g = divmod(c, NCORES // B)
        w = wpk[g]
        in_maps.append({
            "x8": xpk[b][0],
            "xr": xpk[b][1],
            "wq8": w["wq"][0], "wqr": w["wq"][1],
            "wk8": w["wk"][0], "wkr": w["wk"][1],
            "wv8": w["wv"][0], "wvr": w["wv"][1],
            "wo8": w["wo"][0], "wor": w["wo"][1],
            "cos2": cos2,
            "sin2": sin2,
            "maskR": mstrip.astype(ndt),
        })
    return in_maps


_PROGRAM_CACHE = {}


def get_program(T=T_FULL):
    key = (T, MM_DT)
    if key not in _PROGRAM_CACHE:
        _PROGRAM_CACHE[key] = build_program(T)
    return _PROGRAM_CACHE[key]


LAST_EXEC_NS = None


def kernel(x, w_q, w_k, w_v, w_o, mask=None, trace=False):
    """Full inputs in, full output out.  Shards over 8 NeuronCores."""
    global LAST_EXEC_NS
    x = np.asarray(x, dtype=np.float32)
    in_maps = make_in_maps(x, w_q, w_k, w_v, w_o, T_FULL)
    nc = get_program(T_FULL)
    try:
        res = run_bass_kernel_spmd(nc, in_maps, core_ids=list(range(NCORES)),
                                   trace=trace)
    except ModuleNotFoundError:
        # no NTFF profiling hook in this container; run untraced
        res = run_bass_kernel_spmd(nc, in_maps, core_ids=list(range(NCORES)),
                                   trace=False)
    LAST_EXEC_NS = res.exec_time_ns
    gpb = NCORES // B
    out = np.stack([
        sum(np.asarray(res.results[b * gpb + g]["out"], dtype=np.float64)
            for g in range(gpb)).astype(np.float32)
        for b in range(B)
    ])
    return out



# revision 59
# speedup vs baseline: 1.0052x; 1.0014x over previous
"""Trainium2 Bass kernel for CirculantMultiHeadAttention.

Strategy
--------
Host side: the block-circulant weights (4,4,512) are materialized into dense
(2048,2048) matrices, because on TRN2 a dense matmul on the PE array beats any
FFT formulation by a wide margin (the FFT's pointwise stage would swamp the
vector engines).  Work is sharded over the 8 NeuronCores as (batch b in {0,1})
x (head-group g in {0..3}, 4 heads each): core c = 4*b + g.  Each core
computes q/k/v projections for its 4 heads, RoPE, causal attention, and a
*partial* output projection (contracting only its own 512 context features).
The host sums the 4 partials per batch.

Device side (per core, one Bass program, SPMD over 8 cores):
  - the big contractions (q/k/v projections over D=2048, output projection
    over 512 ctx features) run as THREE fp8-e4m3 DoubleRow chains per psum:
    W8@x8 + W8@xr + Wr@x8, where W8/x8 are e4m3 quantizations and Wr/xr the
    e4m3-quantized residuals.  DoubleRow contracts 2 k-tiles per matmul at
    0.5 PE cycles/row, so the 3-chain hi/lo split costs 0.75x the bf16
    cycles while the residual cancellation keeps bf16-class accuracy
    (~0.35% max rel err end-to-end vs the 2e-2 budget).  Scale bookkeeping:
    weights x64 (e4m3 normal range), folded back in the fp16 RoPE tables
    (q/k), the ctx eviction x0.25 (v path), and the out eviction x1/1024.
  - S = q.k stays bf16 (a 128-deep hd contraction cannot pair DoubleRow
    k-tiles without a partition-shifting eviction, and single-fp8 operands
    would put ~5% noise on scores); P = exp(S) stays bf16 (a second exp
    pass for a residual would double the ACT-engine load).  PV and the
    denominator path therefore stay bf16 except the PV/outproj operands
    above.
  - attention in scores-transposed layout: S_T[k, q] = k_tile.T @ q_chunk,
    P_T = exp(S_T * scale) on ScalarE, causal masking only on the single
    mixed 128x128 corner of each diagonal tile (the rest is all-valid),
    PV accumulation ctxT[d, q] += v_tile.T @ P_T, denominators as a bf16
    pairwise partial-sum chain on DVE (second level + last-chunk work on
    Pool, which must never touch PSUM on real HW).
  - ctx is evicted normalized (x 0.25/den) to bf16, then quantized to the
    e4m3 hi/lo pair tiles that feed the output-projection DoubleRow chains.
  - software pipeline: chunk-0 projections alone, then chunk i projections
    interleaved ~1:1.7 with chunk i-1 attention, then last-chunk attention
    with output-projection psums as PE filler, then a drain of the last 4
    output tiles in two-tile g0/g1 half-waves (borrowing the idle S/ctx
    psum banks) so heads-0/1 matmuls cover the last ctx-eviction latency.
  - engine budget at 232us: PE 225.6us (97%), DVE ~150, ACT ~155, Pool ~75.
    PE work: proj 123.3 + S 29 + PV 29 + den-free + outproj 41 (all us).
"""

import os
import sys

import numpy as np

for _p in ("/opt/trn_rl_repo", "/root/.axon_site/_ro/trn_rl_repo"):
    if os.path.isdir(_p) and _p not in sys.path:
        sys.path.insert(0, _p)

import ml_dtypes

import concourse.bass as bass
import concourse.tile as tile
from concourse import bacc, bass_isa, mybir
from concourse.bass_utils import run_bass_kernel_spmd

F32 = mybir.dt.float32
F16 = mybir.dt.float16
AF = mybir.ActivationFunctionType

# Problem geometry (hardcoded per spec).
B, T_FULL, D = 2, 2048, 2048
H, HD = 16, 128
NCORES = 8
HG = 4                    # heads per core
FS = HG * HD              # 512 feature dims per core
P = 128                   # partitions
KT = D // P               # 16 contraction tiles for projections
SCALE = 1.0 / float(np.sqrt(HD))
MASKW = 896               # triangular mask strip width: 512 + 3*128

# Matmul operand dtype.  bfloat16: 1 cycle/row at any moving width on the PE
# (fp32r needs >=256-wide or pays 4x), half the DMA/SBUF of fp32.  HW/sim
# end-to-end relative error ~1e-3 vs the 2e-2 budget.  CIRC_MM_DT=float32r
# restores the TF32-like mode.
MM_DT = os.environ.get("CIRC_MM_DT", "bfloat16")

# q/k/v projections run as THREE fp8(e4m3) DoubleRow chains per psum:
#   W8@x8 + W8@xr + Wr@x8   (W8 = e4m3(64*W), Wr = e4m3(64*W - W8),
#                            x8 = e4m3(x),    xr = e4m3(x - x8))
# DoubleRow contracts 2 k-tiles per matmul at 0.5 PE cycles/row, so the
# 3-chain split runs at 0.75x the bf16 cycle cost with bf16-class accuracy
# (hi/lo residual cancellation; verified ~0.3% max rel err end to end).
# The 64x weight scale keeps e4m3 mantissas in the normal range; it is
# divided back out in the RoPE tables (q/k) and in the host-side w_o (v).
WSCALE = 64.0
NPAIR = KT // 2           # 8 DoubleRow k-tile pairs per contraction


def _mm_dt():
    return getattr(mybir.dt, MM_DT)


def _np_dt():
    return ml_dtypes.bfloat16 if MM_DT == "bfloat16" else np.float32


# ---------------------------------------------------------------------------
# Device program
# ---------------------------------------------------------------------------

def _body(es, tc, io, T):
    nc = tc.nc
    ntc = T // 512            # t-chunks of 512
    nkt = T // P              # 128-wide t/k tiles
    mdt = _mm_dt()
    E4 = mybir.dt.float8e4
    DR = mybir.MatmulPerfMode.DoubleRow

    x8d, xrd, wq8d, wqrd, wk8d, wkrd, wv8d, wvrd, \
        wo8d, word, cos2, sin2, maskR, out = io
    OSC = 1.0 / (16.0 * WSCALE)     # ctx 16x * wo 64x, folded at out evict

    # ---- persistent SBUF tiles ------------------------------------------
    const = es.enter_context(tc.tile_pool(name="const", bufs=1))
    mask_sb = const.tile([P, MASKW], mdt, tag="maskR", name="mask_sb")

    # q/k stay SBUF-resident across phases in [feat, t] layout, one tile per
    # head; v in [t, feat] tiles.  No DRAM bounce.
    qkp = es.enter_context(tc.tile_pool(name="qkall", bufs=HG))
    q_all = [qkp.tile([P, T], mdt, tag="qall", name="q_all") for _ in range(HG)]
    k_all = [qkp.tile([P, T], mdt, tag="kall", name="k_all") for _ in range(HG)]
    vap = es.enter_context(tc.tile_pool(name="vall", bufs=nkt))
    v_all = [None] * nkt

    # output-projection operands in fp8 hi/lo pair layout: ctx as two
    # head-pair tiles [P, 2, T] per variant, w_o as [P, 2, D] per pair group
    wop = es.enter_context(tc.tile_pool(name="wo", bufs=4))
    wo8_sb = [wop.tile([P, 2, D], E4, tag="wo", name="wo8_sb")
              for _ in range(2)]
    wor_sb = [wop.tile([P, 2, D], E4, tag="wo", name="wor_sb")
              for _ in range(2)]
    ctxp = es.enter_context(tc.tile_pool(name="ctx", bufs=HG))
    cx8_sb = [ctxp.tile([P, 2, T], E4, tag="ctx", name="cx8_sb")
              for _ in range(2)]
    cxr_sb = [ctxp.tile([P, 2, T], E4, tag="ctx", name="cxr_sb")
              for _ in range(2)]

    with (
        tc.tile_pool(name="wq", bufs=1) as wqp,
        tc.tile_pool(name="wk", bufs=1) as wkp,
        tc.tile_pool(name="wv", bufs=1) as wvp,
        tc.tile_pool(name="xt", bufs=4) as xtp,
        tc.tile_pool(name="pev", bufs=2) as evp,
        tc.tile_pool(name="trig", bufs=2) as trigp,
        tc.tile_pool(name="pT", bufs=8) as pTp,
        tc.tile_pool(name="pacc", bufs=2) as paccp,
        tc.tile_pool(name="amisc", bufs=2) as amp,
        tc.tile_pool(name="oev", bufs=7) as oevp,
        tc.tile_pool(name="pps", bufs=4, space="PSUM") as psp,
        tc.tile_pool(name="sps", bufs=2, space="PSUM") as sps,
        tc.tile_pool(name="cps", bufs=2, space="PSUM") as cps,
    ):
        # ---- input DMAs: x + wv on SP, wq + wo + mask on Pool (gpsimd),
        # cos/sin + wk on Activation, so the v-projection weights never
        # queue behind wk and the PE can start on x[0]/wq[0] immediately.
        # fp8 streams are packed host-side as [128, pair, 2, cols] so one
        # DMA fills a whole chunk/weight tile in DoubleRow layout.
        # first chunk + first weights split in halves so the first psum
        # chain can start on pairs 0-3 while pairs 4-7 are still in flight
        x8_first = xtp.tile([P, NPAIR, 2, 512], E4, tag="xt", name="x8_sb")
        xr_first = xtp.tile([P, NPAIR, 2, 512], E4, tag="xt", name="xr_sb")
        qp = NPAIR // 4
        for q_i in range(4):
            nc.sync.dma_start(out=x8_first[:, q_i * qp:(q_i + 1) * qp],
                              in_=x8d[:, 0, q_i * qp:(q_i + 1) * qp])
        wq_sb = [wqp.tile([P, NPAIR, 2, FS], E4, tag="wq", name="wq_sb",
                          bufs=2) for _ in range(2)]
        wk_sb = [wkp.tile([P, NPAIR, 2, FS], E4, tag="wk", name="wk_sb",
                          bufs=2) for _ in range(2)]
        wv_sb = [wvp.tile([P, NPAIR, 2, FS], E4, tag="wv", name="wv_sb",
                          bufs=2) for _ in range(2)]
        for q_i in range(4):
            nc.gpsimd.dma_start(out=wq_sb[0][:, q_i * qp:(q_i + 1) * qp],
                                in_=wq8d[:, q_i * qp:(q_i + 1) * qp])
        nc.gpsimd.dma_start(out=wq_sb[1][:], in_=wqrd[:])
        trig_sb = [None] * ntc
        cos_sb0 = trigp.tile([P, 512], F16, tag="cos", name="cos_sb")
        nc.scalar.dma_start(out=cos_sb0[:], in_=cos2[:, 0:512])
        sin_sb0 = trigp.tile([P, 512], F16, tag="sin", name="sin_sb")
        nc.scalar.dma_start(out=sin_sb0[:], in_=sin2[:, 0:512])
        trig_sb[0] = (cos_sb0, sin_sb0)
        nc.scalar.dma_start(out=xr_first[:], in_=xrd[:, 0])
        nc.scalar.dma_start(out=wk_sb[0][:], in_=wk8d[:])
        nc.sync.dma_start(out=wk_sb[1][:], in_=wkrd[:])
        nc.sync.dma_start(out=wv_sb[0][:], in_=wv8d[:])
        nc.sync.dma_start(out=wv_sb[1][:], in_=wvrd[:])
        for g in range(2):
            nc.gpsimd.dma_start(out=wo8_sb[g][:], in_=wo8d[g])
            nc.gpsimd.dma_start(out=wor_sb[g][:], in_=word[g])
        nc.gpsimd.dma_start(out=mask_sb[:], in_=maskR[:, :])

        # ---- emitter builders -------------------------------------------
        def proj_chunk_emitters(tci):
            """12 closures: 8 q/k head-projections (fused RoPE) + 4 v."""
            tsl = slice(tci * 512, (tci + 1) * 512)
            if tci == 0:
                x8_sb, xr_sb = x8_first, xr_first
            else:
                x8_sb = xtp.tile([P, NPAIR, 2, 512], E4, tag="xt",
                                 name="x8_sb")
                xr_sb = xtp.tile([P, NPAIR, 2, 512], E4, tag="xt",
                                 name="xr_sb")

            def prefetch():
                if tci > 0:
                    cos_sb = trigp.tile([P, 512], F16, tag="cos",
                                        name="cos_sb")
                    nc.gpsimd.dma_start(out=cos_sb[:], in_=cos2[:, tsl])
                    sin_sb = trigp.tile([P, 512], F16, tag="sin",
                                        name="sin_sb")
                    nc.gpsimd.dma_start(out=sin_sb[:], in_=sin2[:, tsl])
                    trig_sb[tci] = (cos_sb, sin_sb)
                    nc.sync.dma_start(out=x8_sb[:], in_=x8d[:, tci])
                    nc.gpsimd.dma_start(out=xr_sb[:], in_=xrd[:, tci])

            def qk_em(wsb, dst, h):
                def em():
                    cos_sb, sin_sb = trig_sb[tci]
                    hsl = slice(h * P, (h + 1) * P)
                    ps = psp.tile([P, 512], F32, tag="ps", name="ps")
                    chains = ((wsb[0], x8_sb), (wsb[0], xr_sb),
                              (wsb[1], x8_sb))
                    for ci, (wt, xt) in enumerate(chains):
                        for m in range(NPAIR):
                            nc.tensor.matmul(
                                ps[:], wt[:, m, :, hsl], xt[:, m, :, :],
                                start=(ci == 0 and m == 0),
                                stop=(ci == 2 and m == NPAIR - 1),
                                perf_mode=DR)
                    # RoPE: rot = [-odd; even] of ps (ACT, the psum read),
                    # then all-16-bit DVE ops at the 2x_1p rate
                    rot = evp.tile([P, 512], mdt, tag="rot", name="rot")
                    nc.scalar.mul(rot[0:64, :], ps[64:128, :], -1.0)
                    nc.scalar.copy(rot[64:128, :], ps[0:64, :])
                    o = evp.tile([P, 512], mdt, tag="o", name="o")
                    nc.vector.tensor_mul(o[:], ps[:], cos_sb[:])
                    nc.vector.tensor_mul(rot[:], rot[:], sin_sb[:])
                    nc.vector.tensor_add(dst[h][:, tsl], o[:], rot[:])
                return em

            def v_em(ts):
                def em():
                    tt = tci * 4 + ts
                    psl = slice(ts * P, (ts + 1) * P)
                    ps = psp.tile([P, FS], F32, tag="ps", name="ps")
                    chains = ((x8_sb, wv_sb[0]), (xr_sb, wv_sb[0]),
                              (x8_sb, wv_sb[1]))
                    for ci, (xt, wt) in enumerate(chains):
                        for m in range(NPAIR):
                            nc.tensor.matmul(
                                ps[:], xt[:, m, :, psl], wt[:, m, :, :],
                                start=(ci == 0 and m == 0),
                                stop=(ci == 2 and m == NPAIR - 1),
                                perf_mode=DR)
                    vt = vap.tile([P, FS], mdt, tag="vall", name="v_all")
                    nc.vector.tensor_copy(vt[:], ps[:])
                    v_all[tt] = vt
                return em

            ems = []
            for wsb, dst in ((wq_sb, q_all), (wk_sb, k_all)):
                for h in range(HG):
                    ems.append(qk_em(wsb, dst, h))
            for ts in range(4):
                ems.append(v_em(ts))
            return prefetch, ems

        drain_ps = {}

        def outproj_psum(tt, ncj, final=False, evict_act=False, half=None):
            # one psum group of 6 fp8 DoubleRow matmuls (hi*hi + lo*hi +
            # hi*lo chains x 2 head-pair groups, ~640ns of dep-free PE
            # work) -- the filler currency interleaved into the attention.
            # half=0/1 emits only the g=0 / g=1 chain halves (drain waves).
            nsl = slice(ncj * 512, (ncj + 1) * 512)
            tsl = slice(tt * P, (tt + 1) * P)
            if half == 1:
                ps = drain_ps.pop((tt, ncj))
            elif half == 0 and tt % 2 == 1:
                # odd drain tiles borrow the idle S/ctx psum banks so two
                # tiles' g0 waves can run ahead of the last ctx eviction
                pool = sps if ncj < 2 else cps
                ps = pool.tile([P, 512], F32,
                               tag="sps" if ncj < 2 else "cps", name="ops")
            else:
                ps = psp.tile([P, 512], F32, tag="ps", name="ops")
            chains = ((cx8_sb, wo8_sb), (cxr_sb, wo8_sb), (cx8_sb, wor_sb))
            order = [(c, g) for g in range(2) for c in ((0, 2, 1) if g
                                                        else (0, 1, 2))]
            if half is not None:
                order = [(c, g) for c, g in order if g == half]
            for n_i, (ci, g) in enumerate(order):
                cx, wo_v = chains[ci]
                nc.tensor.matmul(ps[:], cx[g][:, :, tsl],
                                 wo_v[g][:, :, nsl],
                                 start=(half in (None, 0) and n_i == 0),
                                 stop=(half in (None, 1) and
                                       n_i == len(order) - 1),
                                 perf_mode=DR)
            if half == 0:
                drain_ps[(tt, ncj)] = ps
                return
            o = oevp.tile([P, 512], mdt, tag="o", name="o")
            if final and ncj % 2:
                nc.scalar.mul(o[:], ps[:], OSC)
            else:
                nc.vector.tensor_scalar_mul(o[:], ps[:], OSC)
            eng = nc.gpsimd if ncj % 2 else nc.sync
            eng.dma_start(out=out[tsl, nsl], in_=o[:])

        def attn_head_emitters(h, qc):
            """nmg+1 closures; micro-group i = S+exp for kt pair i, with
            the masked PV + denominator accumulation pipelined one step
            behind.  PE filler (outproj of tile ftt) is woven in before the
            early S pairs; ftt is shifted back one tile so the h=0 head of
            each chunk fills with a tile whose ctx is long finished."""
            qsl = slice(qc * 512, (qc + 1) * 512)
            nk = 4 * (qc + 1)
            nmg = nk // 2
            hsl = slice(h * P, (h + 1) * P)
            ftt = 4 * (qc - 1) + h - 1
            fillers = [(ftt, j) for j in range(4)] if ftt >= 0 else []
            if qc == ntc - 1 and h == HG - 1:
                # last head also covers tile ftt+1 so the post-attention
                # drain only has 4 tiles left
                fillers += [(ftt + 1, j) for j in range(4)]
            st = {}

            def tile_slices(kt):
                j = kt - 4 * qc
                c0 = 128 * j if j > 0 else 0
                return slice(qc * 512 + c0, (qc + 1) * 512), slice(c0, 512), c0

            def s_pair(i):
                for kt in (2 * i, 2 * i + 1):
                    lsl, psl, c0 = tile_slices(kt)
                    s_ps = sps.tile([P, 512], F32, tag="sps", name="s_ps")
                    nc.tensor.matmul(s_ps[:, psl],
                                     k_all[h][:, kt * P:(kt + 1) * P],
                                     q_all[h][:, lsl], start=True, stop=True)
                    p_t = pTp.tile([P, 512], mdt, tag="pT", name="p_t")
                    nc.scalar.activation(p_t[:, psl], s_ps[:, psl], AF.Exp,
                                         scale=SCALE)
                    st[kt] = (p_t, None)

            def pv_pair(i):
                kts = (2 * i, 2 * i + 1)
                for kt in kts:
                    _, psl, c0 = tile_slices(kt)
                    pt, _ = st[kt]
                    if kt >= 4 * qc:
                        # only the first 128 cols of a diagonal tile mix
                        # valid/invalid; beyond them every row is valid
                        nc.gpsimd.tensor_mul(pt[:, c0:c0 + P],
                                             pt[:, c0:c0 + P],
                                             mask_sb[:, 384:384 + P])
                    nc.tensor.matmul(st["ctx"][:, psl], v_all[kt][:, hsl],
                                     pt[:, psl],
                                     start=(kt == 0), stop=(kt == nk - 1))
                # denominator accumulation (off the PE): full-width pairs
                # stay in a bf16 partial-sum chain at 2x DVE rate; partial
                # tiles and the flush go through the fp32 accumulator
                k0, k1 = kts
                full = k1 < 4 * qc + 1   # both tiles full 512 wide
                pt0, _ = st[k0]
                pt1, _ = st[k1]
                if full and k0 > 0:
                    t1 = pTp.tile([P, 512], mdt, tag="ds", name="t1",
                                  bufs=4)
                    t1eng = nc.gpsimd if qc == ntc - 1 else nc.vector
                    t1eng.tensor_add(t1[:], pt0[:], pt1[:])
                    if st.get("dsum") is None:
                        st["dsum"] = t1
                    else:
                        t2 = pTp.tile([P, 512], mdt, tag="ds", name="t2",
                                      bufs=4)
                        nc.gpsimd.tensor_add(t2[:], st["dsum"][:], t1[:])
                        st["dsum"] = t2
                else:
                    for kt in kts:
                        _, psl, _ = tile_slices(kt)
                        pt, _ = st[kt]
                        if kt == 0:
                            nc.vector.tensor_copy(st["pacc"][:], pt[:])
                        else:
                            nc.vector.tensor_add(st["pacc"][:, psl],
                                                 st["pacc"][:, psl],
                                                 pt[:, psl])
                for kt in kts:
                    del st[kt]

            def em_i(i):
                def em():
                    if i == 0:
                        st["ctx"] = cps.tile([P, 512], F32, tag="cps",
                                             name="ctx_ps")
                        st["pacc"] = paccp.tile([P, 512], F32, tag="pacc",
                                                name="pacc")
                    if i < nmg:
                        if fillers and i < nmg - 1:
                            rem = max(nmg - 1 - i, 1)
                            nf = (len(fillers) + rem - 1) // rem
                            for _ in range(nf):
                                tt_f, j_f = fillers.pop(0)
                                outproj_psum(tt_f, j_f)
                        s_pair(i)
                    if i > 0:
                        pv_pair(i - 1)
                    last = qc == ntc - 1 and h == HG - 1
                    if i == nmg - 1 and last:
                        if st.get("dsum") is not None:
                            # dsum is complete one micro-step early; flush
                            # now to keep it off the drain critical path
                            nc.vector.tensor_add(st["pacc"][:],
                                                 st["pacc"][:],
                                                 st["dsum"][:])
                            st["dsum"] = None
                        # pacc cols [0:256) got their final (pair nmg-2)
                        # contribution already -- the pair nmg-1 diagonal
                        # adds only touch [256:512).  Reduce and recip the
                        # first half early, off the drain critical path.
                        rs_early = amp.tile([P, 512], F32, tag="rs",
                                            name="rs_red")
                        nc.gpsimd.partition_all_reduce(
                            rs_early[:, 0:256], st["pacc"][:, 0:256],
                            channels=P, reduce_op=bass_isa.ReduceOp.add)
                        nc.vector.reciprocal(rs_early[:, 0:256],
                                             rs_early[:, 0:256])
                        st["rs_early"] = rs_early
                    if i == nmg:
                        if st.get("dsum") is not None:
                            nc.vector.tensor_add(st["pacc"][:],
                                                 st["pacc"][:],
                                                 st["dsum"][:])
                            st["dsum"] = None
                        if st.get("rs_early") is not None:
                            rs_red = st.pop("rs_early")
                            nc.gpsimd.partition_all_reduce(
                                rs_red[:, 256:512], st["pacc"][:, 256:512],
                                channels=P, reduce_op=bass_isa.ReduceOp.add)
                            nc.vector.reciprocal(rs_red[:, 256:512],
                                                 rs_red[:, 256:512])
                        else:
                            rs_red = amp.tile([P, 512], F32, tag="rs",
                                              name="rs_red")
                            nc.gpsimd.partition_all_reduce(
                                rs_red[:], st["pacc"][:], channels=P,
                                reduce_op=bass_isa.ReduceOp.add)
                            nc.vector.reciprocal(rs_red[:], rs_red[:])
                        # cbf = 0.25 * ctx_psum / den  (16x true scale --
                        # keeps the e4m3 hi part clear of the 240 ceiling)
                        g, blk = divmod(h, 2)
                        # for the very last head, evict in two half-width
                        # pipelined pieces so the output-projection drain
                        # can start on the first piece sooner
                        parts = ((slice(0, 256), slice(qc * 512,
                                                       qc * 512 + 256)),
                                 (slice(256, 512), slice(qc * 512 + 256,
                                                         (qc + 1) * 512))
                                 ) if last else ((slice(0, 512), qsl),)
                        cbf = amp.tile([P, 512], mdt, tag="cbf", name="cbf")
                        for psl_c, qsl_c in parts:
                            if last:
                                # write the e4m3 hi tile straight from the
                                # normalize op -- the drain's first chains
                                # need only cx8, so this drops the bf16 +
                                # Pool-copy hop from the critical path
                                nc.vector.scalar_tensor_tensor(
                                    cx8_sb[g][:, blk, qsl_c],
                                    st["ctx"][:, psl_c], 0.25,
                                    rs_red[:, psl_c],
                                    mybir.AluOpType.mult,
                                    mybir.AluOpType.mult)
                            nc.vector.scalar_tensor_tensor(
                                cbf[:, psl_c], st["ctx"][:, psl_c], 0.25,
                                rs_red[:, psl_c],
                                mybir.AluOpType.mult, mybir.AluOpType.mult)
                            if not last:
                                nc.gpsimd.tensor_copy(
                                    cx8_sb[g][:, blk, qsl_c], cbf[:, psl_c])
                            sube = nc.vector if last else nc.gpsimd
                            sube.tensor_sub(cxr_sb[g][:, blk, qsl_c],
                                            cbf[:, psl_c],
                                            cx8_sb[g][:, blk, qsl_c])
                return em
            return [em_i(i) for i in range(nmg + 1)]

        # ---- schedule ----------------------------------------------------
        # software pipeline: chunk-0 projections alone, then chunk tci's
        # projections interleaved with chunk tci-1's attention, then the
        # last chunk's attention alone.  This spreads the attention's
        # DVE/ACT load (exp, masks, denominators) across the whole
        # timeline instead of saturating those engines after the
        # projections finish.
        pfs, emss = [], []
        for tci in range(ntc):
            pf, ems = proj_chunk_emitters(tci)
            pfs.append(pf)
            emss.append(ems)
        for i, em in enumerate(emss[0]):
            if i == 4:
                pfs[1]()    # chunk-1 x/trig DMAs fire during chunk 0
            em()
        for tci in range(1, ntc):
            pe_ems = emss[tci]
            at_ems = [em for h in range(HG)
                      for em in attn_head_emitters(h, tci - 1)]
            npe, na = len(pe_ems), len(at_ems)
            ipe = ia = 0
            while ipe < npe or ia < na:
                if ipe < npe and (ia >= na or ipe * na <= ia * npe):
                    if ipe == 6 and tci + 1 < ntc:
                        pfs[tci + 1]()
                    pe_ems[ipe]()
                    ipe += 1
                else:
                    at_ems[ia]()
                    ia += 1
        for h in range(HG):
            for em in attn_head_emitters(h, ntc - 1):
                em()
        # final output projection: tiles 12..15 in two-tile waves of
        # g0-then-g1 halves so up to 8 psums of head-0/1 work run while
        # the last head's ctx eviction is still in flight
        t0 = 4 * (ntc - 1)
        for ta in (t0, t0 + 2):
            for tt in (ta, ta + 1):
                for ncj in range(4):
                    outproj_psum(tt, ncj, final=True, half=0)
            for tt in (ta, ta + 1):
                for ncj in range(4):
                    outproj_psum(tt, ncj, final=True, half=1)


def build_program(T=T_FULL):
    from contextlib import ExitStack

    nc = bacc.Bacc("TRN2", target_bir_lowering=False, debug=False,
                   num_devices=NCORES)
    mdt = _mm_dt()
    E4 = mybir.dt.float8e4
    ntc = T // 512
    x8d = nc.dram_tensor("x8", (P, ntc, NPAIR, 2, 512), E4,
                         kind="ExternalInput").ap()
    xrd = nc.dram_tensor("xr", (P, ntc, NPAIR, 2, 512), E4,
                         kind="ExternalInput").ap()
    wts = {}
    for wn in ("wq8", "wqr", "wk8", "wkr", "wv8", "wvr"):
        wts[wn] = nc.dram_tensor(wn, (P, NPAIR, 2, FS), E4,
                                 kind="ExternalInput").ap()
    wo8d = nc.dram_tensor("wo8", (2, P, 2, D), E4, kind="ExternalInput").ap()
    word = nc.dram_tensor("wor", (2, P, 2, D), E4, kind="ExternalInput").ap()
    cos2 = nc.dram_tensor("cos2", (P, T), F16, kind="ExternalInput").ap()
    sin2 = nc.dram_tensor("sin2", (P, T), F16, kind="ExternalInput").ap()
    maskR = nc.dram_tensor("maskR", (P, MASKW), mdt,
                           kind="ExternalInput").ap()
    out = nc.dram_tensor("out", (T, D), mdt, kind="ExternalOutput").ap()

    io = (x8d, xrd, wts["wq8"], wts["wqr"], wts["wk8"], wts["wkr"],
          wts["wv8"], wts["wvr"], wo8d, word, cos2, sin2, maskR, out)
    with tile.TileContext(nc) as tc:
        with ExitStack() as es:
            _body(es, tc, io, T)
    nc.compile()
    return nc


# ---------------------------------------------------------------------------
# Host-side data prep
# ---------------------------------------------------------------------------

def dense_from_circulant(w):
    """(qb, pb, bs) generating vectors -> dense (qb*bs, pb*bs) matrix."""
    w = np.asarray(w, dtype=np.float32)
    qb, pb, bs = w.shape
    idx = (np.arange(bs)[:, None] - np.arange(bs)[None, :]) % bs
    blocks = w[:, :, idx]                      # (qb, pb, bs, bs)
    return np.ascontiguousarray(
        blocks.transpose(0, 2, 1, 3).reshape(qb * bs, pb * bs))


_EO_PERM = np.concatenate([np.arange(0, HD, 2), np.arange(1, HD, 2)])


def _perm_rows_even_odd(w_rows):
    """Permute each 128-row head block to (even rows, odd rows)."""
    nh = w_rows.shape[0] // HD
    blocks = w_rows.reshape(nh, HD, -1)[:, _EO_PERM, :]
    return blocks.reshape(w_rows.shape)


def rope_tables(T=T_FULL, theta=10000.0):
    # 1/WSCALE folds the fp8 weight scale back out of the q/k psums
    inv = 1.0 / (theta ** (np.arange(0, HD, 2, dtype=np.float32) / HD))
    ang = np.arange(T, dtype=np.float32)[:, None] * inv[None, :]
    cos = (np.cos(ang) / WSCALE).astype(np.float16).T      # (64, T)
    sin = (np.sin(ang) / WSCALE).astype(np.float16).T
    cos2 = np.ascontiguousarray(np.concatenate([cos, cos], axis=0))
    sin2 = np.ascontiguousarray(np.concatenate([sin, sin], axis=0))
    return cos2, sin2


E4NP = ml_dtypes.float8_e4m3


def _pack_x_fp8(xT):
    """xT (D, T) fp32 -> (x8, xr) packed [128, ntc, NPAIR, 2, 512] e4m3."""
    x8 = xT.astype(E4NP)
    xr = (xT - x8.astype(np.float32)).astype(E4NP)
    ntc = xT.shape[1] // 512

    def pack(a):
        # d = 256*m + 128*b + p ; t = 512*c + j
        a = a.reshape(NPAIR, 2, P, ntc, 512).transpose(2, 3, 0, 1, 4)
        return np.ascontiguousarray(a)
    return pack(x8), pack(xr)


def _pack_w_fp8(wT):
    """wT (D, FS) fp32 -> (w8, wr) packed [128, NPAIR, 2, FS] e4m3."""
    ws = wT * WSCALE
    w8 = ws.astype(E4NP)
    wr = (ws - w8.astype(np.float32)).astype(E4NP)

    def pack(a):
        a = a.reshape(NPAIR, 2, P, FS).transpose(2, 0, 1, 3)
        return np.ascontiguousarray(a)
    return pack(w8), pack(wr)


def _pack_wo_fp8(woT):
    """woT (FS, D) fp32 -> (wo8, wor) packed [2, 128, 2, D] e4m3.
    Pair group g holds heads (2g, 2g+1) as DoubleRow contraction subtiles."""
    ws = woT * WSCALE
    w8 = ws.astype(E4NP)
    wr = (ws - w8.astype(np.float32)).astype(E4NP)

    def pack(a):
        a = a.reshape(2, 2, P, D).transpose(0, 2, 1, 3)
        return np.ascontiguousarray(a)
    return pack(w8), pack(wr)


def mask_strip():
    kk = np.arange(P)[:, None]
    c = np.arange(MASKW)[None, :]
    return np.ascontiguousarray(((c - 384) >= kk).astype(np.float32))


def make_in_maps(x, w_q, w_k, w_v, w_o, T=T_FULL):
    """Build the 8 per-core input maps from full inputs."""
    ndt = _np_dt()
    x = np.asarray(x, dtype=np.float32)
    Wq = dense_from_circulant(w_q)
    Wk = dense_from_circulant(w_k)
    Wv = dense_from_circulant(w_v)
    Wo = dense_from_circulant(w_o)
    cos2, sin2 = rope_tables(T)
    mstrip = mask_strip()

    xpk = [_pack_x_fp8(np.ascontiguousarray(x[b, :T, :].T))
           for b in range(B)]
    wpk = {}
    for g in range(NCORES // B):
        fs = slice(FS * g, FS * (g + 1))
        wpk[g] = {
            "wq": _pack_w_fp8(
                np.ascontiguousarray(_perm_rows_even_odd(Wq[fs, :]).T)),
            "wk": _pack_w_fp8(
                np.ascontiguousarray(_perm_rows_even_odd(Wk[fs, :]).T)),
            "wv": _pack_w_fp8(np.ascontiguousarray(Wv[fs, :].T)),
            "wo": _pack_wo_fp8(np.ascontiguousarray(Wo[:, fs].T)),
        }
    in_maps = []
    for c in range(NCORES):
        b, g = divmod(c, NCORES // B)
        w = wpk[g]
        in_maps.append({
            "x8": xpk[b][0],
            "xr": xpk[b][1],
            "wq8": w["wq"][0], "wqr": w["wq"][1],
            "wk8": w["wk"][0], "wkr": w["wk"][1],
            "wv8": w["wv"][0], "wvr": w["wv"][1],
            "wo8": w["wo"][0], "wor": w["wo"][1],
            "cos2": cos2,
            "sin2": sin2,
            "maskR": mstrip.astype(ndt),
        })
    return in_maps


_PROGRAM_CACHE = {}


def get_program(T=T_FULL):
    key = (T, MM_DT)
    if key not in _PROGRAM_CACHE:
        _PROGRAM_CACHE[key] = build_program(T)
    return _PROGRAM_CACHE[key]


LAST_EXEC_NS = None


def kernel(x, w_q, w_k, w_v, w_o, mask=None, trace=False):
    """Full inputs in, full output out.  Shards over 8 NeuronCores."""
    global LAST_EXEC_NS
    x = np.asarray(x, dtype=np.float32)
    in_maps = make_in_maps(x, w_q, w_k, w_v, w_o, T_FULL)
    nc = get_program(T_FULL)
    try:
        res = run_bass_kernel_spmd(nc, in_maps, core_ids=list(range(NCORES)),
                                   trace=trace)
    except ModuleNotFoundError:
        # no NTFF profiling hook in this container; run untraced
        res = run_bass_kernel_spmd(nc, in_maps, core_ids=list(range(NCORES)),
                                   trace=False)
    LAST_EXEC_NS = res.exec_time_ns
    gpb = NCORES // B
    out = np.stack([
        sum(np.asarray(res.results[b * gpb + g]["out"], dtype=np.float64)
            for g in range(gpb)).astype(np.float32)
        for b in range(B)
    ])
    return out

